# revision 1
# baseline (speedup 1.0000x reference)
"""2-layer GCN (GCNConv -> ReLU -> GCNConv -> Sigmoid) on 8 TRN2 NeuronCores.

Strategy (dst-node sharding, 8 cores):
  - Nodes sharded by destination range: core c owns dst rows [c*NPC, (c+1)*NPC).
  - Fold the symmetric normalization into per-node scales:
        out_d = sigmoid(dinv_d * (A0 @ (dinv*relu(dinv*(A0 @ (dinv*x@W1)) ...)))...
    so the sparse aggregation A0 (unweighted multi-adjacency + self loops)
    acts on 50-wide "scaled" tables and no per-edge weight is needed.
  - Per layer: z table (node-major, bf16, rows padded to 256B) is AllGathered;
    each core gathers z[src] for its edges with dma_gather (int16 indices ->
    4 gathers against 2-core table ranges), reduces 128-edge chunks with
    one-hot S1 matmuls (S1 built on-device by DVE is_equal vs iota), and
    scatter-adds per-chunk partial sums into fp16 dst accumulators with
    dma_scatter_add (conflict-free by (range, chunk-parity) regions split
    across two accumulator arrays).
  - Epilogues apply dinv scales/bias/activation on ACT, and the tiny W2 matmul
    runs per dst block after a PE transpose.

Host side does only index/metadata preprocessing (sorting edges, degree
counts, chunk layout) and input re-layout (x transposed + bf16).
"""

import os
import numpy as np
import ml_dtypes

import concourse.bass as bass
import concourse.bacc as bacc
import concourse.tile as tile
import concourse.mybir as mybir
from concourse.bass_utils import run_bass_kernel_spmd

BF16 = mybir.dt.bfloat16
FP16 = mybir.dt.float16
F32 = mybir.dt.float32
I16 = mybir.dt.int16

C = 8        # cores
P = 128      # partitions
SLOT = 32    # dst slots per chunk (chunk spans < 32 dst nodes)
DEAD = SLOT  # col_rel value marking a dead (padded) edge


def _cfg_for(n_nodes, fin, hid, out_dim, ch_r, gb):
    npc = n_nodes // C
    nb = -(-npc // P)
    npcp = nb * P
    kt = -(-fin // P)
    cfg = dict(
        N=n_nodes, FIN=fin, HID=hid, OUT=out_dim,
        NPC=npc, NB=nb, NPCP=npcp, KT=kt, KP=kt * P,
        RN=2 * npcp,                  # rows per gather range (2 cores)
        TBL=C * npcp,                 # allgathered table rows
        CH_R=ch_r,                    # chunks per (core, range), uniform
        GB=gb,                        # gather batch tokens
        BR=(ch_r * P) // gb,          # gather batches per range
        TPR=ch_r // 8,                # partial tiles per (range, parity) region
        ARR=npcp + P,                 # accumulator rows (+dummy block)
    )
    assert cfg["BR"] * gb == ch_r * P and ch_r % 8 == 0 and gb % 128 == 0
    return cfg


# ----------------------------------------------------------------- host prep

def _preprocess(x, edge_index, W1, b1, W2, b2):
    N, FIN = x.shape
    HID = W1.shape[1]
    OUT = W2.shape[1]
    assert N % C == 0
    NPC = N // C
    NB = -(-NPC // P)
    NPCP = NB * P
    RN = 2 * NPCP

    row = edge_index[0].astype(np.int64)
    col = edge_index[1].astype(np.int64)
    loops = np.arange(N, dtype=np.int64)
    rows = np.concatenate([row, loops])
    cols = np.concatenate([col, loops])

    deg = np.bincount(cols, minlength=N).astype(np.float32)
    dinv = (1.0 / np.sqrt(deg.astype(np.float64))).astype(np.float32)

    # table row of node n in the allgathered (row-padded) table
    tbl_row = (rows // NPC) * NPCP + (rows % NPC)
    src_range = tbl_row // RN
    idx_local = (tbl_row - src_range * RN).astype(np.int64)
    core = cols // NPC
    col_local = (cols - core * NPC).astype(np.int64)

    order = np.lexsort((col_local, src_range, core))
    core_s = core[order]
    rng_s = src_range[order]
    coll_s = col_local[order]
    idxl_s = idx_local[order]

    # chunk every (core, range) segment: break at 128 tokens or dst span 32
    bounds_all = {}
    max_chunks = 0
    seg_edges = {}
    for c in range(C):
        c_end = np.searchsorted(core_s, c + 1)
        c_start = np.searchsorted(core_s, c)
        for r in range(4):
            s0 = c_start + np.searchsorted(rng_s[c_start:c_end], r)
            s1 = c_start + np.searchsorted(rng_s[c_start:c_end], r + 1)
            seg_edges[(c, r)] = (s0, s1)
            cseg = coll_s[s0:s1]
            bounds = []
            i = 0
            n = len(cseg)
            while i < n:
                j = int(np.searchsorted(cseg, cseg[i] + SLOT, side="left"))
                j = min(j, i + P, n)
                bounds.append((i, j))
                i = j
            bounds_all[(c, r)] = bounds
            max_chunks = max(max_chunks, len(bounds))
    ch_r = max(64, ((max_chunks + 63) // 64) * 64)
    # dma_gather/dma_scatter_add are limited to 1024 indices per instruction
    # (SWDGE descriptor-ring capacity; >1024 wedges the device).
    gb = 1024
    cfg = _cfg_for(N, FIN, HID, OUT, ch_r, gb)
    CH_R, GB, BR, TPR = cfg["CH_R"], cfg["GB"], cfg["BR"], cfg["TPR"]
    CHUNKS = 4 * CH_R
    DUMMY = NPCP  # dummy dst row in accumulator arrays

    # weights / tables, shared across cores
    KP = cfg["KP"]
    xt = np.zeros((KP, C * NPCP), dtype=ml_dtypes.bfloat16)
    xtf = np.ascontiguousarray(x.T).astype(ml_dtypes.bfloat16)
    for c in range(C):
        xt[:FIN, c * NPCP:c * NPCP + NPC] = xtf[:, c * NPC:(c + 1) * NPC]
    w1 = np.zeros((KP, 64), dtype=ml_dtypes.bfloat16)
    w1[:FIN, :HID] = W1.astype(ml_dtypes.bfloat16)
    w2 = np.zeros((64, OUT), dtype=ml_dtypes.bfloat16)
    w2[:HID, :] = W2.astype(ml_dtypes.bfloat16)
    iota32 = np.tile(np.arange(SLOT, dtype=np.float32), (P, 1)).astype(ml_dtypes.bfloat16)
    ident = np.eye(P, dtype=np.float32).astype(ml_dtypes.bfloat16)
    b1r = np.zeros((1, 64), np.float32)
    b1r[0, :HID] = b1
    b2r = b2.reshape(1, OUT).astype(np.float32)
    has_b1 = bool(np.any(b1))
    has_b2 = bool(np.any(b2))

    in_maps = []
    for c in range(C):
        gidx = np.zeros((4 * BR, P, GB // 16), np.int16)
        colrel_tile = np.full((P, CHUNKS), float(DEAD), np.float32)
        sidx = np.full((8, P, (TPR * P) // 16), DUMMY, np.int64)

        for r in range(4):
            s0, s1 = seg_edges[(c, r)]
            cseg = coll_s[s0:s1]
            iseg = idxl_s[s0:s1]
            bounds = bounds_all[(c, r)]
            gtok = np.zeros((CH_R, P), np.int64)
            crel = np.full((CH_R, P), DEAD, np.int64)
            sreg = np.full((2, TPR * P), DUMMY, np.int64)  # per parity
            for j, (s, e) in enumerate(bounds):
                L = e - s
                gtok[j, :L] = iseg[s:e]
                if L < P:
                    gtok[j, L:] = iseg[e - 1]
                cr = cseg[s:e] - cseg[s]
                crel[j, :L] = cr
                # scatter slots: chunk j -> region (r, j%2), tile (j%8)//... :
                pi = j % 2
                tr = j // 8
                q = (j % 8) // 2
                slots = tr * P + q * SLOT + cr
                sreg[pi][slots] = cseg[s] + cr + 0  # dst local row
            # assemble per-core tensors
            colrel_tile[:, r * CH_R:(r + 1) * CH_R] = crel.T
            for bi in range(BR):
                toks = gtok[bi * (GB // P):(bi + 1) * (GB // P)].reshape(-1)
                gidx[r * BR + bi] = np.tile(
                    toks.reshape(GB // 16, 16).T, (8, 1))
            for pi in range(2):
                sidx[2 * r + pi] = np.tile(
                    sreg[pi].reshape((TPR * P) // 16, 16).T, (8, 1))

        nb = cfg["NB"]
        dloc = np.ones(NPCP, np.float32)
        dloc[:NPC] = dinv[c * NPC:(c + 1) * NPC]
        dinv_pp = dloc.reshape(nb, P).T.copy()          # [128, NB]
        dinv2_pp = (dloc * dloc).reshape(nb, P).T.copy()
        sqdloc = np.ones(NPCP, np.float32)
        sqdloc[:NPC] = np.sqrt(deg[c * NPC:(c + 1) * NPC])

        m = {
            "xt": np.ascontiguousarray(xt[:, c * NPCP:(c + 1) * NPCP]),
            "w1": w1, "w2": w2, "iota32": iota32, "ident": ident,
            "colrel": colrel_tile.astype(ml_dtypes.bfloat16),
            "gidx": gidx.astype(np.int16),
            "sidx": sidx.astype(np.int16),
            "dinv_pp": dinv_pp, "dinv2_pp": dinv2_pp,
            "b1f": np.tile(b1r, (P, 1)), "b2r": b2r,
            "sqd": sqdloc.reshape(1, NPCP),
            "sqd_pp": sqdloc.reshape(nb, P).T.copy(),
        }
        in_maps.append(m)

    cfg["HAS_B1"] = has_b1
    cfg["HAS_B2"] = has_b2
    return cfg, in_maps


# ------------------------------------------------------------- program build

def _build_program(cfg, phases="full"):
    NB, KT, NPCP, RN, TBL = cfg["NB"], cfg["KT"], cfg["NPCP"], cfg["RN"], cfg["TBL"]
    CH_R, GB, BR, TPR, ARR = cfg["CH_R"], cfg["GB"], cfg["BR"], cfg["TPR"], cfg["ARR"]
    OUT = cfg["OUT"]
    CHUNKS = 4 * CH_R
    SPB = GB // P      # chunk slots per gather batch
    NPAIR = SPB // 8   # psum-tile pairs per batch

    nc = bacc.Bacc("TRN2", target_bir_lowering=False, debug=False, num_devices=C)

    xt_d = nc.dram_tensor("xt", [cfg["KP"], NPCP], BF16, kind="ExternalInput")
    w1_d = nc.dram_tensor("w1", [cfg["KP"], 64], BF16, kind="ExternalInput")
    w2_d = nc.dram_tensor("w2", [64, OUT], BF16, kind="ExternalInput")
    iota_d = nc.dram_tensor("iota32", [P, SLOT], BF16, kind="ExternalInput")
    ident_d = nc.dram_tensor("ident", [P, P], BF16, kind="ExternalInput")
    colrel_d = nc.dram_tensor("colrel", [P, CHUNKS], BF16, kind="ExternalInput")
    gidx_d = nc.dram_tensor("gidx", [4 * BR, P, GB // 16], I16, kind="ExternalInput")
    sidx_d = nc.dram_tensor("sidx", [8, P, (TPR * P) // 16], I16, kind="ExternalInput")
    dinv_d = nc.dram_tensor("dinv_pp", [P, NB], F32, kind="ExternalInput")
    dinv2_d = nc.dram_tensor("dinv2_pp", [P, NB], F32, kind="ExternalInput")
    b1f_d = nc.dram_tensor("b1f", [P, 64], F32, kind="ExternalInput")
    b2_d = nc.dram_tensor("b2r", [1, OUT], F32, kind="ExternalInput")
    sqd_d = nc.dram_tensor("sqd", [1, NPCP], F32, kind="ExternalInput")
    sqdpp_d = nc.dram_tensor("sqd_pp", [P, NB], F32, kind="ExternalInput")
    out_d = nc.dram_tensor("out", [NPCP, OUT], F32, kind="ExternalOutput")
    dbg_d = None
    if phases != "full":
        dbg_d = nc.dram_tensor("dbg", [TBL, P], F32, kind="ExternalOutput")

    with tile.TileContext(nc) as tc:
        with tc.tile_pool(name="const", bufs=1) as cpool, \
             tc.tile_pool(name="sb", bufs=2) as sb, \
             tc.tile_pool(name="parts", bufs=4) as parts_pool, \
             tc.tile_pool(name="psum", bufs=2, space="PSUM") as pp, \
             tc.tile_pool(name="dram", bufs=1, space="DRAM") as dram:

            # ---- constants
            w1_t = cpool.tile([P, KT, 64], BF16)
            nc.sync.dma_start(
                out=w1_t[:], in_=w1_d.ap().rearrange("(k p) e -> p k e", p=P))
            w2_t = cpool.tile([64, OUT], BF16)
            nc.sync.dma_start(out=w2_t[:], in_=w2_d.ap())
            iota_t = cpool.tile([P, SLOT], BF16)
            nc.sync.dma_start(out=iota_t[:], in_=iota_d.ap())
            ident_t = cpool.tile([P, P], BF16)
            nc.sync.dma_start(out=ident_t[:], in_=ident_d.ap())
            colrel_t = cpool.tile([P, CHUNKS], BF16)
            nc.sync.dma_start(out=colrel_t[:], in_=colrel_d.ap())
            dinv_t = cpool.tile([P, NB], F32)
            nc.sync.dma_start(out=dinv_t[:], in_=dinv_d.ap())
            dinv2_t = cpool.tile([P, NB], F32)
            nc.sync.dma_start(out=dinv2_t[:], in_=dinv2_d.ap())
            b2_t = cpool.tile([1, OUT], F32)
            nc.sync.dma_start(out=b2_t[:], in_=b2_d.ap())
            sqd_t = None
            if cfg["HAS_B2"]:
                sqd_t = cpool.tile([1, NPCP], F32)
                nc.sync.dma_start(out=sqd_t[:], in_=sqd_d.ap())
            b1f_t = None
            sqdpp_t = None
            if cfg["HAS_B1"]:
                b1f_t = cpool.tile([P, 64], F32)
                nc.sync.dma_start(out=b1f_t[:], in_=b1f_d.ap())
                sqdpp_t = cpool.tile([P, NB], F32)
                nc.sync.dma_start(out=sqdpp_t[:], in_=sqdpp_d.ap())

            zz = cpool.tile([P, 4096], BF16)
            nc.vector.memset(zz[:], 0.0)

            # ---- DRAM temporaries
            z_loc = dram.tile([NPCP, P], BF16)
            zfull = dram.tile([TBL, P], BF16)
            h1_loc = dram.tile([NPCP, P], BF16)
            h1full = dram.tile([TBL, P], BF16)
            arrA = dram.tile([ARR, P], FP16)
            arrB = dram.tile([ARR, P], FP16)

            def zero_dram(t, rows, dt):
                src = zz[:].bitcast(dt) if dt != BF16 else zz[:]
                off = 0
                while off < rows:
                    n = min(4096, rows - off)
                    nc.sync.dma_start(
                        out=t[:][off:off + n, :].rearrange(
                            "(p a) e -> p (a e)", p=P),
                        in_=src[:, :n],
                    )
                    off += n

            zero_dram(z_loc, NPCP, BF16)
            zero_dram(h1_loc, NPCP, BF16)
            zero_dram(arrA, ARR, FP16)
            zero_dram(arrB, ARR, FP16)

            # ---- phase Z: z_loc = dinv * (x @ W1), node-major bf16
            for b in range(NB):
                xtt = sb.tile([P, KT, P], BF16, tag="xtt")
                nc.sync.dma_start(
                    out=xtt[:],
                    in_=xt_d.ap()[:, b * P:(b + 1) * P].rearrange(
                        "(k p) n -> p k n", p=P))
                psz = pp.tile([P, 64], F32, tag="psz")
                for k in range(KT):
                    nc.tensor.matmul(
                        out=psz[:], lhsT=xtt[:, k, :], rhs=w1_t[:, k, :],
                        start=(k == 0), stop=(k == KT - 1))
                zst = sb.tile([P, 64], BF16, tag="zst")
                nc.scalar.activation(
                    out=zst[:], in_=psz[:],
                    func=mybir.ActivationFunctionType.Copy,
                    scale=dinv_t[:, b:b + 1])
                nc.sync.dma_start(
                    out=z_loc[:][b * P:(b + 1) * P, 0:64], in_=zst[:])

            nc.gpsimd.collective_compute(
                "AllGather", mybir.AluOpType.bypass,
                replica_groups=[list(range(C))],
                ins=[z_loc.opt()], outs=[zfull.opt()])

            def dump_dbg(src, rows, row_off=0):
                nc.gpsimd.dma_start(
                    out=dbg_d.ap()[row_off:row_off + rows, :], in_=src[:][:rows, :])

            # ---- aggregation layer (shared for L1/L2)
            def agg_layer(src_full, n_ranges=4, do_scatter=True,
                          dump_parts=False):
                for r in range(n_ranges):
                    part_even = parts_pool.tile([P, TPR, 64], FP16, tag="parts")
                    part_odd = parts_pool.tile([P, TPR, 64], FP16, tag="parts")
                    partials = [part_even, part_odd]
                    for bi in range(BR):
                        bidx = r * BR + bi
                        gixt = sb.tile([P, GB // 16], I16, tag="gixt")
                        nc.sync.dma_start(out=gixt[:], in_=gidx_d.ap()[bidx])
                        msg = sb.tile([P, SPB, P], BF16, tag="msg")
                        nc.gpsimd.dma_gather(
                            out_ap=msg[:],
                            in_ap=src_full[:][r * RN:(r + 1) * RN, :],
                            idxs_ap=gixt[:],
                            num_idxs=GB, num_idxs_reg=GB, elem_size=P)
                        s1t = sb.tile([P, SPB, SLOT], BF16, tag="s1t")
                        cb = bidx * SPB
                        nc.vector.tensor_tensor(
                            out=s1t[:],
                            in0=iota_t[:][:, None, :].to_broadcast([P, SPB, SLOT]),
                            in1=colrel_t[:, cb:cb + SPB][:, :, None].to_broadcast(
                                [P, SPB, SLOT]),
                            op=mybir.AluOpType.is_equal)
                        for pair in range(NPAIR):
                            ps_e = pp.tile([P, 64], F32, tag="pse")
                            ps_o = pp.tile([P, 64], F32, tag="pso")
                            ps = [ps_e, ps_o]
                            for jj in range(8):
                                cl = pair * 8 + jj
                                q = (jj // 2)
                                nc.tensor.matmul(
                                    out=ps[jj % 2][SLOT * q:SLOT * (q + 1), :],
                                    lhsT=s1t[:, cl, :],
                                    rhs=msg[:, cl, 0:64],
                                    start=True, stop=True,
                                    tile_position=(0, SLOT * q),
                                    skip_group_check=True)
                            tr = bi * NPAIR + pair
                            for pi in range(2):
                                if tr % 2 == 0:
                                    nc.vector.tensor_copy(
                                        out=partials[pi][:, tr, :], in_=ps[pi][:])
                                else:
                                    nc.scalar.copy(
                                        out=partials[pi][:, tr, :], in_=ps[pi][:])
                    if dump_parts and r == 0:
                        for pi in range(2):
                            nc.gpsimd.dma_start(
                                out=dbg_d.ap()[pi * TPR * P:(pi + 1) * TPR * P, 0:64]
                                .rearrange("(t p) e -> p t e", p=P),
                                in_=partials[pi][:])
                    if not do_scatter:
                        continue
                    n_sub = max(1, (TPR * P) // 1024)
                    tps = TPR // n_sub  # tiles per sub-scatter
                    for pi in range(2):
                        sxt = sb.tile([P, (TPR * P) // 16], I16, tag="sxt")
                        nc.sync.dma_start(out=sxt[:], in_=sidx_d.ap()[2 * r + pi])
                        arr = arrA if pi == 0 else arrB
                        for s in range(n_sub):
                            nc.gpsimd.dma_scatter_add(
                                out_ap=arr[:][:, 0:64],
                                in_ap=partials[pi][:, s * tps:(s + 1) * tps, :],
                                idxs_ap=sxt[:, s * (tps * P // 16):(s + 1) * (tps * P // 16)],
                                num_idxs=tps * P, num_idxs_reg=tps * P,
                                elem_size=64, elem_step=P)

            if phases in ("l1g", "l1r0"):
                agg_layer(zfull, n_ranges=1, do_scatter=(phases == "l1r0"),
                          dump_parts=(phases == "l1g"))
                if phases == "l1r0":
                    dump_dbg(arrA, ARR)
                    dump_dbg(arrB, ARR, ARR)
                lvl = 0
            else:
                order = ["z", "l1", "ep1", "l2", "full"]
                lvl = order.index(phases)

            if lvl >= 1:
                agg_layer(zfull)
            if phases == "z":
                dump_dbg(zfull, TBL)
            if phases == "l1":
                dump_dbg(arrA, ARR)
                dump_dbg(arrB, ARR, ARR)

            # ---- L1 epilogue: h1_loc = relu(dinv^2*agg [+ dinv*b1]) (scaled h1)
            for b in range(NB if lvl >= 2 else 0):
                at = sb.tile([P, 64], FP16, tag="at")
                nc.sync.dma_start(out=at[:], in_=arrA[:][b * P:(b + 1) * P, 0:64])
                bt = sb.tile([P, 64], FP16, tag="bt")
                nc.sync.dma_start(out=bt[:], in_=arrB[:][b * P:(b + 1) * P, 0:64])
                st = sb.tile([P, 64], F32, tag="st")
                nc.vector.tensor_tensor(
                    out=st[:], in0=at[:], in1=bt[:], op=mybir.AluOpType.add)
                if cfg["HAS_B1"]:
                    # st += sqrt(deg) * b1 (per-partition scalar x row vector)
                    tmp = sb.tile([P, 64], F32, tag="tmpb")
                    nc.vector.tensor_scalar_mul(
                        tmp[:], b1f_t[:], sqdpp_t[:, b:b + 1])
                    nc.vector.tensor_tensor(
                        out=st[:], in0=st[:], in1=tmp[:], op=mybir.AluOpType.add)
                h1t = sb.tile([P, 64], BF16, tag="h1t")
                nc.scalar.activation(
                    out=h1t[:], in_=st[:],
                    func=mybir.ActivationFunctionType.Relu,
                    scale=dinv2_t[:, b:b + 1])
                nc.sync.dma_start(
                    out=h1_loc[:][b * P:(b + 1) * P, 0:64], in_=h1t[:])

            if lvl >= 2:
                zero_dram(arrA, ARR, FP16)
                zero_dram(arrB, ARR, FP16)
                nc.gpsimd.collective_compute(
                    "AllGather", mybir.AluOpType.bypass,
                    replica_groups=[list(range(C))],
                    ins=[h1_loc.opt()], outs=[h1full.opt()])
            if phases == "ep1":
                dump_dbg(h1full, TBL)
            if lvl >= 3:
                agg_layer(h1full)
            if phases == "l2":
                dump_dbg(arrA, ARR)
                dump_dbg(arrB, ARR, ARR)

            # ---- L2 epilogue: out = sigmoid(dinv * (agg2 @ W2) [+ b2])
            for b in range(NB if lvl >= 4 else 0):
                at = sb.tile([P, 64], FP16, tag="at")
                nc.sync.dma_start(out=at[:], in_=arrA[:][b * P:(b + 1) * P, 0:64])
                bt = sb.tile([P, 64], FP16, tag="bt")
                nc.sync.dma_start(out=bt[:], in_=arrB[:][b * P:(b + 1) * P, 0:64])
                st = sb.tile([P, 64], BF16, tag="st2")
                nc.vector.tensor_tensor(
                    out=st[:], in0=at[:], in1=bt[:], op=mybir.AluOpType.add)
                tp = pp.tile([64, P], BF16, tag="psz")
                nc.tensor.transpose(out=tp[:], in_=st[:], identity=ident_t[:])
                zt = sb.tile([64, P], BF16, tag="zt")
                nc.scalar.copy(out=zt[:], in_=tp[:])
                ps3 = pp.tile([P, OUT], F32, tag="pse")
                nc.tensor.matmul(
                    out=ps3[:], lhsT=zt[:], rhs=w2_t[:],
                    start=True, stop=not cfg["HAS_B2"],
                    skip_group_check=True)
                if cfg["HAS_B2"]:
                    nc.tensor.matmul(
                        out=ps3[:], lhsT=sqd_t[:, b * P:(b + 1) * P],
                        rhs=b2_t[:], start=False, stop=True,
                        skip_group_check=True)
                ot = sb.tile([P, OUT], F32, tag="ot")
                nc.scalar.activation(
                    out=ot[:], in_=ps3[:],
                    func=mybir.ActivationFunctionType.Sigmoid,
                    scale=dinv_t[:, b:b + 1])
                nc.sync.dma_start(out=out_d.ap()[b * P:(b + 1) * P, :], in_=ot[:])

    nc.compile()
    return nc


_PROGRAM_CACHE = {}
LAST_EXEC_NS = None
LAST_TRACE = None


def _get_program(cfg):
    key = tuple(sorted((k, v) for k, v in cfg.items()))
    if key not in _PROGRAM_CACHE:
        _PROGRAM_CACHE[key] = _build_program(cfg)
    return _PROGRAM_CACHE[key]


def kernel(x, edge_index, W1, b1, W2, b2):
    x = np.asarray(x, np.float32)
    edge_index = np.asarray(edge_index)
    W1 = np.asarray(W1, np.float32)
    b1 = np.asarray(b1, np.float32)
    W2 = np.asarray(W2, np.float32)
    b2 = np.asarray(b2, np.float32)

    cfg, in_maps = _preprocess(x, edge_index, W1, b1, W2, b2)
    nc = _get_program(cfg)
    trace = bool(os.environ.get("KERNEL_TRACE"))
    res = run_bass_kernel_spmd(nc, in_maps, core_ids=list(range(C)), trace=trace)
    global LAST_EXEC_NS, LAST_TRACE
    if res.exec_time_ns:
        LAST_EXEC_NS = res.exec_time_ns
        LAST_TRACE = res
    NPC, OUT = cfg["NPC"], cfg["OUT"]
    out = np.empty((cfg["N"], OUT), np.float32)
    for c in range(C):
        out[c * NPC:(c + 1) * NPC] = res.results[c]["out"][:NPC]
    return out



# revision 3
# speedup vs baseline: 2.6686x; 2.6686x over previous
"""2-layer GCN (GCNConv -> ReLU -> GCNConv -> Sigmoid) on 8 TRN2 NeuronCores.

Strategy (dst-node sharding, 8 cores):
  - Nodes sharded by destination range: core c owns dst rows [c*NPC, (c+1)*NPC).
  - Fold the symmetric normalization into per-node scales:
        out_d = sigmoid(dinv_d * (A0 @ (dinv*relu(dinv*(A0 @ (dinv*x@W1)) ...)))...
    so the sparse aggregation A0 (unweighted multi-adjacency) acts on 50-wide
    "scaled" tables and no per-edge weight is needed.  Self-loops are NOT sent
    through the edge machinery; their contribution (the node's own scaled row)
    is added locally in each layer epilogue.
  - Per layer: z table (node-major, bf16, rows padded to 256B) is AllGathered
    into a Shared scratchpad; each core gathers z[src] for its edges with
    dma_gather (int16 indices -> 4 gathers against 2-core table ranges),
    reduces 128-edge chunks with one-hot S1 matmuls (S1 built on-device by DVE
    is_equal vs iota), and scatter-adds per-chunk partial sums into fp16 dst
    accumulators with dma_scatter_add (conflict-free by (range, chunk-parity)
    regions split across two accumulator arrays).
  - SWDGE work is spread over 4 queues: gathers round-robin queues 0/1 so
    descriptor prep overlaps ring drain; arrA scatters serialize on queue 2,
    arrB scatters on queue 3 (scatters into the same array must stay ordered
    to avoid concurrent read-modify-write on shared dst rows).
  - Epilogues apply dinv scales/bias/activation on ACT, and the tiny W2 matmul
    runs per dst block after a PE transpose.

Host side does only index/metadata preprocessing (sorting edges, degree
counts, chunk layout) and input re-layout (x transposed + bf16).
"""

import os
import numpy as np
import ml_dtypes

import concourse.bass as bass
import concourse.bacc as bacc
import concourse.tile as tile
import concourse.mybir as mybir
from concourse.bass_utils import run_bass_kernel_spmd

BF16 = mybir.dt.bfloat16
FP16 = mybir.dt.float16
F32 = mybir.dt.float32
I16 = mybir.dt.int16

C = 8        # cores
P = 128      # partitions
SLOT = 32    # dst slots per chunk (chunk spans < 32 dst nodes)
DEAD = SLOT  # col_rel value marking a dead (padded) edge


def _cfg_for(n_nodes, fin, hid, out_dim, ch_r, gb):
    npc = n_nodes // C
    nb = -(-npc // P)
    npcp = nb * P
    kt = -(-fin // P)
    cfg = dict(
        N=n_nodes, FIN=fin, HID=hid, OUT=out_dim,
        NPC=npc, NB=nb, NPCP=npcp, KT=kt, KP=kt * P,
        RN=2 * npcp,                  # rows per gather range (2 cores)
        TBL=C * npcp,                 # allgathered table rows
        CH_R=ch_r,                    # chunks per (core, range), uniform
        GB=gb,                        # gather batch tokens
        BR=(ch_r * P) // gb,          # gather batches per range
        TPR=ch_r // 8,                # partial tiles per (range, parity) region
        ARR=npcp + P,                 # accumulator rows (+dummy block)
    )
    assert cfg["BR"] * gb == ch_r * P and ch_r % 8 == 0 and gb % 128 == 0
    return cfg


# ----------------------------------------------------------------- host prep

def _preprocess(x, edge_index, W1, b1, W2, b2):
    N, FIN = x.shape
    HID = W1.shape[1]
    OUT = W2.shape[1]
    assert N % C == 0
    NPC = N // C
    NB = -(-NPC // P)
    NPCP = NB * P
    RN = 2 * NPCP

    rows = edge_index[0].astype(np.int64)
    cols = edge_index[1].astype(np.int64)

    # degree includes the self-loop (GCNConv add_self_loops=True)
    deg = (np.bincount(cols, minlength=N) + 1).astype(np.float32)
    dinv = (1.0 / np.sqrt(deg.astype(np.float64))).astype(np.float32)

    # table row of node n in the allgathered (row-padded) table
    tbl_row = (rows // NPC) * NPCP + (rows % NPC)
    src_range = tbl_row // RN
    idx_local = (tbl_row - src_range * RN).astype(np.int64)
    core = cols // NPC
    col_local = (cols - core * NPC).astype(np.int64)

    order = np.lexsort((col_local, src_range, core))
    core_s = core[order]
    rng_s = src_range[order]
    coll_s = col_local[order]
    idxl_s = idx_local[order]

    # chunk every (core, range) segment: break at 128 tokens or dst span 32
    bounds_all = {}
    max_chunks = 0
    seg_edges = {}
    for c in range(C):
        c_end = np.searchsorted(core_s, c + 1)
        c_start = np.searchsorted(core_s, c)
        for r in range(4):
            s0 = c_start + np.searchsorted(rng_s[c_start:c_end], r)
            s1 = c_start + np.searchsorted(rng_s[c_start:c_end], r + 1)
            seg_edges[(c, r)] = (s0, s1)
            cseg = coll_s[s0:s1]
            bounds = []
            i = 0
            n = len(cseg)
            while i < n:
                j = int(np.searchsorted(cseg, cseg[i] + SLOT, side="left"))
                j = min(j, i + P, n)
                bounds.append((i, j))
                i = j
            bounds_all[(c, r)] = bounds
            max_chunks = max(max_chunks, len(bounds))
    ch_r = max(64, ((max_chunks + 7) // 8) * 8)
    # dma_gather/dma_scatter_add are limited to 1024 indices per instruction
    # (SWDGE descriptor-ring capacity; >1024 wedges the device).
    gb = 1024
    cfg = _cfg_for(N, FIN, HID, OUT, ch_r, gb)
    CH_R, GB, BR, TPR = cfg["CH_R"], cfg["GB"], cfg["BR"], cfg["TPR"]
    CHUNKS = 4 * CH_R
    DUMMY = NPCP  # dummy dst row in accumulator arrays

    # weights / tables, shared across cores
    KP = cfg["KP"]
    w1 = np.zeros((KP, 64), dtype=ml_dtypes.bfloat16)
    w1[:FIN, :HID] = W1.astype(ml_dtypes.bfloat16)
    w2 = np.zeros((64, OUT), dtype=ml_dtypes.bfloat16)
    w2[:HID, :] = W2.astype(ml_dtypes.bfloat16)
    iota32 = np.tile(np.arange(SLOT, dtype=np.float32), (P, 1)).astype(ml_dtypes.bfloat16)
    ident = np.eye(P, dtype=np.float32).astype(ml_dtypes.bfloat16)
    b1r = np.zeros((1, 64), np.float32)
    b1r[0, :HID] = b1
    b2r = b2.reshape(1, OUT).astype(np.float32)
    has_b1 = bool(np.any(b1))
    has_b2 = bool(np.any(b2))

    in_maps = []
    for c in range(C):
        gidx = np.zeros((4 * BR, P, GB // 16), np.int16)
        colrel_tile = np.full((P, CHUNKS), float(DEAD), np.float32)
        sidx = np.full((8, P, (TPR * P) // 16), DUMMY, np.int64)

        for r in range(4):
            s0, s1 = seg_edges[(c, r)]
            cseg = coll_s[s0:s1]
            iseg = idxl_s[s0:s1]
            bounds = bounds_all[(c, r)]
            gtok = np.zeros((CH_R, P), np.int64)
            crel = np.full((CH_R, P), DEAD, np.int64)
            sreg = np.full((2, TPR * P), DUMMY, np.int64)  # per parity
            for j, (s, e) in enumerate(bounds):
                L = e - s
                gtok[j, :L] = iseg[s:e]
                if L < P:
                    gtok[j, L:] = iseg[e - 1]
                cr = cseg[s:e] - cseg[s]
                crel[j, :L] = cr
                # scatter slots: chunk j -> region (r, j%2), tile (j%8)//... :
                pi = j % 2
                tr = j // 8
                q = (j % 8) // 2
                slots = tr * P + q * SLOT + cr
                sreg[pi][slots] = cseg[s] + cr + 0  # dst local row
            # assemble per-core tensors
            colrel_tile[:, r * CH_R:(r + 1) * CH_R] = crel.T
            for bi in range(BR):
                toks = gtok[bi * (GB // P):(bi + 1) * (GB // P)].reshape(-1)
                gidx[r * BR + bi] = np.tile(
                    toks.reshape(GB // 16, 16).T, (8, 1))
            for pi in range(2):
                sidx[2 * r + pi] = np.tile(
                    sreg[pi].reshape((TPR * P) // 16, 16).T, (8, 1))

        nb = cfg["NB"]
        dloc = np.ones(NPCP, np.float32)
        dloc[:NPC] = dinv[c * NPC:(c + 1) * NPC]
        dinv_pp = dloc.reshape(nb, P).T.copy()          # [128, NB]
        dinv2_pp = (dloc * dloc).reshape(nb, P).T.copy()
        sqdloc = np.ones(NPCP, np.float32)
        sqdloc[:NPC] = np.sqrt(deg[c * NPC:(c + 1) * NPC])

        # x slice for this core, laid out so each phase-Z block load is
        # contiguous per partition: xt[b, p, k*128 + i] = x[base + b*128 + i,
        # k*128 + p]  (partition p = feature-within-ktile, free = (k, node)).
        xc = x[c * NPC:(c + 1) * NPC].astype(ml_dtypes.bfloat16)
        xpad = np.zeros((NPCP, KP), dtype=ml_dtypes.bfloat16)
        xpad[:NPC, :FIN] = xc
        xt = np.ascontiguousarray(
            xpad.reshape(NB, P, cfg["KT"], P).transpose(0, 3, 2, 1)
        ).reshape(NB, P, cfg["KT"] * P)

        m = {
            "xt": xt,
            "w1": w1, "w2": w2, "iota32": iota32, "ident": ident,
            "colrel": colrel_tile.astype(ml_dtypes.bfloat16),
            "gidx": gidx.astype(np.int16),
            "sidx": sidx.astype(np.int16),
            "dinv_pp": dinv_pp, "dinv2_pp": dinv2_pp,
            "b1f": np.tile(b1r, (P, 1)), "b2r": b2r,
            "sqd": sqdloc.reshape(1, NPCP),
            "sqd_pp": sqdloc.reshape(nb, P).T.copy(),
        }
        in_maps.append(m)

    cfg["HAS_B1"] = has_b1
    cfg["HAS_B2"] = has_b2
    return cfg, in_maps


# ------------------------------------------------------------- program build

def _build_program(cfg):
    NB, KT, NPCP, RN, TBL = cfg["NB"], cfg["KT"], cfg["NPCP"], cfg["RN"], cfg["TBL"]
    CH_R, GB, BR, TPR, ARR = cfg["CH_R"], cfg["GB"], cfg["BR"], cfg["TPR"], cfg["ARR"]
    OUT = cfg["OUT"]
    CHUNKS = 4 * CH_R
    SPB = GB // P      # chunk slots per gather batch
    NPAIR = SPB // 8   # psum-tile pairs per batch
    SREG = TPR * P     # scatter indices per (range, parity) region

    nc = bacc.Bacc("TRN2", target_bir_lowering=False, debug=False,
                   num_devices=C, num_swdge_queues=4)

    xt_d = nc.dram_tensor("xt", [NB, P, KT * P], BF16, kind="ExternalInput")
    w1_d = nc.dram_tensor("w1", [cfg["KP"], 64], BF16, kind="ExternalInput")
    w2_d = nc.dram_tensor("w2", [64, OUT], BF16, kind="ExternalInput")
    iota_d = nc.dram_tensor("iota32", [P, SLOT], BF16, kind="ExternalInput")
    ident_d = nc.dram_tensor("ident", [P, P], BF16, kind="ExternalInput")
    colrel_d = nc.dram_tensor("colrel", [P, CHUNKS], BF16, kind="ExternalInput")
    gidx_d = nc.dram_tensor("gidx", [4 * BR, P, GB // 16], I16, kind="ExternalInput")
    sidx_d = nc.dram_tensor("sidx", [8, P, SREG // 16], I16, kind="ExternalInput")
    dinv_d = nc.dram_tensor("dinv_pp", [P, NB], F32, kind="ExternalInput")
    dinv2_d = nc.dram_tensor("dinv2_pp", [P, NB], F32, kind="ExternalInput")
    b1f_d = nc.dram_tensor("b1f", [P, 64], F32, kind="ExternalInput")
    b2_d = nc.dram_tensor("b2r", [1, OUT], F32, kind="ExternalInput")
    sqd_d = nc.dram_tensor("sqd", [1, NPCP], F32, kind="ExternalInput")
    sqdpp_d = nc.dram_tensor("sqd_pp", [P, NB], F32, kind="ExternalInput")
    out_d = nc.dram_tensor("out", [NPCP, OUT], F32, kind="ExternalOutput")

    with tile.TileContext(nc) as tc:
        with tc.tile_pool(name="const", bufs=1) as cpool, \
             tc.tile_pool(name="sb", bufs=3) as sb, \
             tc.tile_pool(name="parts", bufs=4) as parts_pool, \
             tc.tile_pool(name="psum", bufs=2, space="PSUM") as pp, \
             tc.tile_pool(name="dram", bufs=1, space="DRAM") as dram:

            # ---- constants
            w1_t = cpool.tile([P, KT, 64], BF16)
            nc.sync.dma_start(
                out=w1_t[:], in_=w1_d.ap().rearrange("(k p) e -> p k e", p=P))
            w2_t = cpool.tile([64, OUT], BF16)
            nc.sync.dma_start(out=w2_t[:], in_=w2_d.ap())
            iota_t = cpool.tile([P, SLOT], BF16)
            nc.sync.dma_start(out=iota_t[:], in_=iota_d.ap())
            ident_t = cpool.tile([P, P], BF16)
            nc.sync.dma_start(out=ident_t[:], in_=ident_d.ap())
            colrel_t = cpool.tile([P, CHUNKS], BF16)
            nc.sync.dma_start(out=colrel_t[:], in_=colrel_d.ap())
            dinv_t = cpool.tile([P, NB], F32)
            nc.sync.dma_start(out=dinv_t[:], in_=dinv_d.ap())
            dinv2_t = cpool.tile([P, NB], F32)
            nc.sync.dma_start(out=dinv2_t[:], in_=dinv2_d.ap())
            b2_t = cpool.tile([1, OUT], F32)
            nc.sync.dma_start(out=b2_t[:], in_=b2_d.ap())
            # all gather/scatter index tables live in SBUF for the whole run
            gixt_all = cpool.tile([P, 4 * BR, GB // 16], I16)
            nc.sync.dma_start(
                out=gixt_all[:],
                in_=gidx_d.ap().rearrange("b p g -> p b g"))
            sxt_all = cpool.tile([P, 8, SREG // 16], I16)
            nc.sync.dma_start(
                out=sxt_all[:],
                in_=sidx_d.ap().rearrange("r p g -> p r g"))
            sqd_t = None
            if cfg["HAS_B2"]:
                sqd_t = cpool.tile([1, NPCP], F32)
                nc.sync.dma_start(out=sqd_t[:], in_=sqd_d.ap())
            b1f_t = None
            sqdpp_t = None
            if cfg["HAS_B1"]:
                b1f_t = cpool.tile([P, 64], F32)
                nc.sync.dma_start(out=b1f_t[:], in_=b1f_d.ap())
                sqdpp_t = cpool.tile([P, NB], F32)
                nc.sync.dma_start(out=sqdpp_t[:], in_=sqdpp_d.ap())

            zz = cpool.tile([P, 4096], BF16)
            nc.vector.memset(zz[:], 0.0)

            # ---- DRAM temporaries
            z_loc = dram.tile([NPCP, P], BF16)
            zfull = dram.tile([TBL, P], BF16, addr_space="Shared")
            h1_loc = dram.tile([NPCP, P], BF16)
            h1full = dram.tile([TBL, P], BF16, addr_space="Shared")
            arrA = dram.tile([ARR, P], FP16)
            arrB = dram.tile([ARR, P], FP16)

            def zero_dram(t, rows, dt):
                src = zz[:].bitcast(dt) if dt != BF16 else zz[:]
                off = 0
                while off < rows:
                    n = min(4096, rows - off)
                    nc.sync.dma_start(
                        out=t[:][off:off + n, :].rearrange(
                            "(p a) e -> p (a e)", p=P),
                        in_=src[:, :n],
                    )
                    off += n

            zero_dram(arrA, ARR, FP16)
            zero_dram(arrB, ARR, FP16)

            # ---- phase Z: z_loc = dinv * (x @ W1), node-major bf16
            for b in range(NB):
                xtt = sb.tile([P, KT, P], BF16, tag="xtt")
                nc.sync.dma_start(out=xtt[:], in_=xt_d.ap()[b])
                psz = pp.tile([P, 64], F32, tag="psz")
                for k in range(KT):
                    nc.tensor.matmul(
                        out=psz[:], lhsT=xtt[:, k, :], rhs=w1_t[:, k, :],
                        start=(k == 0), stop=(k == KT - 1))
                zst = sb.tile([P, 64], BF16, tag="zst")
                nc.scalar.activation(
                    out=zst[:], in_=psz[:],
                    func=mybir.ActivationFunctionType.Copy,
                    scale=dinv_t[:, b:b + 1])
                nc.sync.dma_start(
                    out=z_loc[:][b * P:(b + 1) * P, 0:64], in_=zst[:])

            nc.gpsimd.collective_compute(
                "AllGather", mybir.AluOpType.bypass,
                replica_groups=[list(range(C))],
                ins=[z_loc.opt()], outs=[zfull.opt()])

            # ---- aggregation layer (shared for L1/L2)
            def agg_layer(src_full):
                for r in range(4):
                    part_even = parts_pool.tile([P, TPR, 64], FP16, tag="parts")
                    part_odd = parts_pool.tile([P, TPR, 64], FP16, tag="parts")
                    partials = [part_even, part_odd]
                    for bi in range(BR):
                        bidx = r * BR + bi
                        msg = sb.tile([P, SPB, P], BF16, tag="msg")
                        nc.gpsimd.dma_gather(
                            out_ap=msg[:],
                            in_ap=src_full[:][r * RN:(r + 1) * RN, :],
                            idxs_ap=gixt_all[:, bidx, :],
                            num_idxs=GB, num_idxs_reg=GB, elem_size=P,
                            queue_num=bi % 2)
                        s1t = sb.tile([P, SPB, SLOT], BF16, tag="s1t")
                        cb = bidx * SPB
                        nc.vector.tensor_tensor(
                            out=s1t[:],
                            in0=iota_t[:][:, None, :].to_broadcast([P, SPB, SLOT]),
                            in1=colrel_t[:, cb:cb + SPB][:, :, None].to_broadcast(
                                [P, SPB, SLOT]),
                            op=mybir.AluOpType.is_equal)
                        for pair in range(NPAIR):
                            ps_e = pp.tile([P, 64], F32, tag="pse")
                            ps_o = pp.tile([P, 64], F32, tag="pso")
                            ps = [ps_e, ps_o]
                            for jj in range(8):
                                cl = pair * 8 + jj
                                q = (jj // 2)
                                nc.tensor.matmul(
                                    out=ps[jj % 2][SLOT * q:SLOT * (q + 1), :],
                                    lhsT=s1t[:, cl, :],
                                    rhs=msg[:, cl, 0:64],
                                    start=True, stop=True,
                                    tile_position=(0, SLOT * q),
                                    skip_group_check=True)
                            tr = bi * NPAIR + pair
                            for pi in range(2):
                                if tr % 2 == 0:
                                    nc.vector.tensor_copy(
                                        out=partials[pi][:, tr, :], in_=ps[pi][:])
                                else:
                                    nc.scalar.copy(
                                        out=partials[pi][:, tr, :], in_=ps[pi][:])
                    # scatter-add this range's partial tiles (<=1024 idx per
                    # call; ragged tail allowed).  All scatters into the same
                    # accumulator array stay on one queue so their RMWs never
                    # run concurrently.
                    for pi in range(2):
                        arr = arrA if pi == 0 else arrB
                        off = 0
                        while off < SREG:
                            n = min(1024, SREG - off)
                            nc.gpsimd.dma_scatter_add(
                                out_ap=arr[:][:, 0:64],
                                in_ap=partials[pi][:, off // P:(off + n) // P, :],
                                idxs_ap=sxt_all[:, 2 * r + pi,
                                                off // 16:(off + n) // 16],
                                num_idxs=n, num_idxs_reg=n,
                                elem_size=64, elem_step=P,
                                queue_num=2 + pi)
                            off += n

            agg_layer(zfull)

            # ---- L1 epilogue:
            #   h1_loc = relu(dinv^2*(agg + z_self) [+ dinv*b1]) (scaled h1)
            for b in range(NB):
                at = sb.tile([P, 64], FP16, tag="at")
                nc.sync.dma_start(out=at[:], in_=arrA[:][b * P:(b + 1) * P, 0:64])
                bt = sb.tile([P, 64], FP16, tag="bt")
                nc.sync.dma_start(out=bt[:], in_=arrB[:][b * P:(b + 1) * P, 0:64])
                zlt = sb.tile([P, 64], BF16, tag="zlt")
                nc.sync.dma_start(
                    out=zlt[:], in_=z_loc[:][b * P:(b + 1) * P, 0:64])
                st = sb.tile([P, 64], F32, tag="st")
                nc.vector.tensor_tensor(
                    out=st[:], in0=at[:], in1=bt[:], op=mybir.AluOpType.add)
                nc.vector.tensor_tensor(
                    out=st[:], in0=st[:], in1=zlt[:], op=mybir.AluOpType.add)
                if cfg["HAS_B1"]:
                    # st += sqrt(deg) * b1 (per-partition scalar x row vector)
                    tmp = sb.tile([P, 64], F32, tag="tmpb")
                    nc.vector.tensor_scalar_mul(
                        tmp[:], b1f_t[:], sqdpp_t[:, b:b + 1])
                    nc.vector.tensor_tensor(
                        out=st[:], in0=st[:], in1=tmp[:], op=mybir.AluOpType.add)
                h1t = sb.tile([P, 64], BF16, tag="h1t")
                nc.scalar.activation(
                    out=h1t[:], in_=st[:],
                    func=mybir.ActivationFunctionType.Relu,
                    scale=dinv2_t[:, b:b + 1])
                nc.sync.dma_start(
                    out=h1_loc[:][b * P:(b + 1) * P, 0:64], in_=h1t[:])

            zero_dram(arrA, ARR, FP16)
            zero_dram(arrB, ARR, FP16)
            nc.gpsimd.collective_compute(
                "AllGather", mybir.AluOpType.bypass,
                replica_groups=[list(range(C))],
                ins=[h1_loc.opt()], outs=[h1full.opt()])

            agg_layer(h1full)

            # ---- L2 epilogue: out = sigmoid(dinv * ((agg2+h_self) @ W2) [+ b2])
            for b in range(NB):
                at = sb.tile([P, 64], FP16, tag="at")
                nc.sync.dma_start(out=at[:], in_=arrA[:][b * P:(b + 1) * P, 0:64])
                bt = sb.tile([P, 64], FP16, tag="bt")
                nc.sync.dma_start(out=bt[:], in_=arrB[:][b * P:(b + 1) * P, 0:64])
                hlt = sb.tile([P, 64], BF16, tag="hlt")
                nc.sync.dma_start(
                    out=hlt[:], in_=h1_loc[:][b * P:(b + 1) * P, 0:64])
                s0 = sb.tile([P, 64], F32, tag="s0")
                nc.vector.tensor_tensor(
                    out=s0[:], in0=at[:], in1=bt[:], op=mybir.AluOpType.add)
                st = sb.tile([P, 64], BF16, tag="st2")
                nc.vector.tensor_tensor(
                    out=st[:], in0=s0[:], in1=hlt[:], op=mybir.AluOpType.add)
                tp = pp.tile([64, P], BF16, tag="psz")
                nc.tensor.transpose(out=tp[:], in_=st[:], identity=ident_t[:])
                zt = sb.tile([64, P], BF16, tag="zt")
                nc.scalar.copy(out=zt[:], in_=tp[:])
                ps3 = pp.tile([P, OUT], F32, tag="pse")
                nc.tensor.matmul(
                    out=ps3[:], lhsT=zt[:], rhs=w2_t[:],
                    start=True, stop=not cfg["HAS_B2"],
                    skip_group_check=True)
                if cfg["HAS_B2"]:
                    nc.tensor.matmul(
                        out=ps3[:], lhsT=sqd_t[:, b * P:(b + 1) * P],
                        rhs=b2_t[:], start=False, stop=True,
                        skip_group_check=True)
                ot = sb.tile([P, OUT], F32, tag="ot")
                nc.scalar.activation(
                    out=ot[:], in_=ps3[:],
                    func=mybir.ActivationFunctionType.Sigmoid,
                    scale=dinv_t[:, b:b + 1])
                nc.sync.dma_start(out=out_d.ap()[b * P:(b + 1) * P, :], in_=ot[:])

    nc.compile()
    return nc


_PROGRAM_CACHE = {}
LAST_EXEC_NS = None
LAST_TRACE = None


def _get_program(cfg):
    key = tuple(sorted((k, v) for k, v in cfg.items()))
    if key not in _PROGRAM_CACHE:
        _PROGRAM_CACHE[key] = _build_program(cfg)
    return _PROGRAM_CACHE[key]


def kernel(x, edge_index, W1, b1, W2, b2):
    x = np.asarray(x, np.float32)
    edge_index = np.asarray(edge_index)
    W1 = np.asarray(W1, np.float32)
    b1 = np.asarray(b1, np.float32)
    W2 = np.asarray(W2, np.float32)
    b2 = np.asarray(b2, np.float32)

    cfg, in_maps = _preprocess(x, edge_index, W1, b1, W2, b2)
    nc = _get_program(cfg)
    trace = bool(os.environ.get("KERNEL_TRACE"))
    res = run_bass_kernel_spmd(nc, in_maps, core_ids=list(range(C)), trace=trace)
    global LAST_EXEC_NS, LAST_TRACE
    if res.exec_time_ns:
        LAST_EXEC_NS = res.exec_time_ns
        LAST_TRACE = res
    NPC, OUT = cfg["NPC"], cfg["OUT"]
    out = np.empty((cfg["N"], OUT), np.float32)
    for c in range(C):
        out[c * NPC:(c + 1) * NPC] = res.results[c]["out"][:NPC]
    return out


# revision 9
# speedup vs baseline: 2.7440x; 1.0282x over previous
"""2-layer GCN (GCNConv -> ReLU -> GCNConv -> Sigmoid) on 8 TRN2 NeuronCores.

Strategy (dst-node sharding, 8 cores):
  - Nodes sharded by destination range: core c owns dst rows [c*NPC, (c+1)*NPC).
  - Fold the symmetric normalization into per-node scales:
        out_d = sigmoid(dinv_d * (A0 @ (dinv*relu(dinv*(A0 @ (dinv*x@W1)) ...)))...
    so the sparse aggregation A0 (unweighted multi-adjacency) acts on 50-wide
    "scaled" tables and no per-edge weight is needed.  Self-loops are NOT sent
    through the edge machinery; their contribution (the node's own scaled row)
    is added locally in each layer epilogue.
  - Per layer: z table (node-major, bf16, rows padded to 256B) is AllGathered
    into a Shared scratchpad; each core gathers z[src] for its edges with
    dma_gather (int16 indices -> 4 gathers against 2-core table ranges),
    reduces 128-edge chunks with one-hot S1 matmuls (S1 built on-device by DVE
    is_equal vs iota), and scatter-adds per-chunk partial sums into fp16 dst
    accumulators with dma_scatter_add (conflict-free by (range, chunk-parity)
    regions split across two accumulator arrays).
  - SWDGE work is spread over 4 queues: gathers round-robin queues 0/1 so
    descriptor prep overlaps ring drain; arrA scatters serialize on queue 2,
    arrB scatters on queue 3 (scatters into the same array must stay ordered
    to avoid concurrent read-modify-write on shared dst rows).
  - Epilogues apply dinv scales/bias/activation on ACT, and the tiny W2 matmul
    runs per dst block after a PE transpose.

Host side does only index/metadata preprocessing (sorting edges, degree
counts, chunk layout) and input re-layout (x transposed + bf16).
"""

import os
import numpy as np
import ml_dtypes

import concourse.bass as bass
import concourse.bacc as bacc
import concourse.tile as tile
import concourse.mybir as mybir
from concourse.bass_utils import run_bass_kernel_spmd

BF16 = mybir.dt.bfloat16
FP16 = mybir.dt.float16
F32 = mybir.dt.float32
I16 = mybir.dt.int16

C = 8        # cores
P = 128      # partitions
SLOT = 32    # dst slots per chunk (chunk spans < 32 dst nodes)
DEAD = SLOT  # col_rel value marking a dead (padded) edge


def _cfg_for(n_nodes, fin, hid, out_dim, ch_r, gb):
    npc = n_nodes // C
    nb = -(-npc // P)
    npcp = nb * P
    kt = -(-fin // P)
    cfg = dict(
        N=n_nodes, FIN=fin, HID=hid, OUT=out_dim,
        NPC=npc, NB=nb, NPCP=npcp, KT=kt, KP=kt * P,
        RN=2 * npcp,                  # rows per gather range (2 cores)
        TBL=C * npcp,                 # allgathered table rows
        CH_R=ch_r,                    # chunks per (core, range), uniform
        GB=gb,                        # gather batch tokens
        BR=(ch_r * P) // gb,          # gather batches per range
        TPR=ch_r // 8,                # partial tiles per (range, parity) region
        ARR=npcp + P,                 # accumulator rows (+dummy block)
    )
    assert cfg["BR"] * gb == ch_r * P and ch_r % 8 == 0 and gb % 128 == 0
    return cfg


# ----------------------------------------------------------------- host prep

def _preprocess(x, edge_index, W1, b1, W2, b2):
    N, FIN = x.shape
    HID = W1.shape[1]
    OUT = W2.shape[1]
    assert N % C == 0
    NPC = N // C
    NB = -(-NPC // P)
    NPCP = NB * P
    RN = 2 * NPCP

    rows = edge_index[0].astype(np.int64)
    cols = edge_index[1].astype(np.int64)

    # degree includes the self-loop (GCNConv add_self_loops=True)
    deg = (np.bincount(cols, minlength=N) + 1).astype(np.float32)
    dinv = (1.0 / np.sqrt(deg.astype(np.float64))).astype(np.float32)

    # table row of node n in the allgathered (row-padded) table
    tbl_row = (rows // NPC) * NPCP + (rows % NPC)
    src_range = tbl_row // RN
    idx_local = (tbl_row - src_range * RN).astype(np.int64)
    core = cols // NPC
    col_local = (cols - core * NPC).astype(np.int64)

    order = np.lexsort((col_local, src_range, core))
    core_s = core[order]
    rng_s = src_range[order]
    coll_s = col_local[order]
    idxl_s = idx_local[order]

    # chunk every (core, range) segment: break at 128 tokens or dst span 32
    bounds_all = {}
    max_chunks = 0
    seg_edges = {}
    for c in range(C):
        c_end = np.searchsorted(core_s, c + 1)
        c_start = np.searchsorted(core_s, c)
        for r in range(4):
            s0 = c_start + np.searchsorted(rng_s[c_start:c_end], r)
            s1 = c_start + np.searchsorted(rng_s[c_start:c_end], r + 1)
            seg_edges[(c, r)] = (s0, s1)
            cseg = coll_s[s0:s1]
            bounds = []
            i = 0
            n = len(cseg)
            while i < n:
                j = int(np.searchsorted(cseg, cseg[i] + SLOT, side="left"))
                j = min(j, i + P, n)
                bounds.append((i, j))
                i = j
            bounds_all[(c, r)] = bounds
            max_chunks = max(max_chunks, len(bounds))
    ch_r = max(64, ((max_chunks + 7) // 8) * 8)
    # dma_gather/dma_scatter_add are limited to 1024 indices per instruction
    # (SWDGE descriptor-ring capacity; >1024 wedges the device).
    gb = 1024
    cfg = _cfg_for(N, FIN, HID, OUT, ch_r, gb)
    CH_R, GB, BR, TPR = cfg["CH_R"], cfg["GB"], cfg["BR"], cfg["TPR"]
    CHUNKS = 4 * CH_R
    DUMMY = NPCP  # dummy dst row in accumulator arrays

    # weights / tables, shared across cores
    KP = cfg["KP"]
    w1 = np.zeros((KP, 64), dtype=ml_dtypes.bfloat16)
    w1[:FIN, :HID] = W1.astype(ml_dtypes.bfloat16)
    w2 = np.zeros((64, OUT), dtype=ml_dtypes.bfloat16)
    w2[:HID, :] = W2.astype(ml_dtypes.bfloat16)
    iota32 = np.tile(np.arange(SLOT, dtype=np.float32), (P, 1)).astype(ml_dtypes.bfloat16)
    ident = np.eye(P, dtype=np.float32).astype(ml_dtypes.bfloat16)
    b1r = np.zeros((1, 64), np.float32)
    b1r[0, :HID] = b1
    b2r = b2.reshape(1, OUT).astype(np.float32)
    has_b1 = bool(np.any(b1))
    has_b2 = bool(np.any(b2))

    in_maps = []
    for c in range(C):
        gidx = np.zeros((4 * BR, P, GB // 16), np.int16)
        colrel_tile = np.full((P, CHUNKS), float(DEAD), np.float32)
        sidx = np.full((8, P, (TPR * P) // 16), DUMMY, np.int64)

        for r in range(4):
            s0, s1 = seg_edges[(c, r)]
            cseg = coll_s[s0:s1]
            iseg = idxl_s[s0:s1]
            bounds = bounds_all[(c, r)]
            gtok = np.zeros((CH_R, P), np.int64)
            crel = np.full((CH_R, P), DEAD, np.int64)
            sreg = np.full((2, TPR * P), DUMMY, np.int64)  # per parity
            for j, (s, e) in enumerate(bounds):
                L = e - s
                gtok[j, :L] = iseg[s:e]
                if L < P:
                    gtok[j, L:] = iseg[e - 1]
                cr = cseg[s:e] - cseg[s]
                crel[j, :L] = cr
                # scatter slots: chunk j -> region (r, j%2), tile (j%8)//... :
                pi = j % 2
                tr = j // 8
                q = (j % 8) // 2
                slots = tr * P + q * SLOT + cr
                sreg[pi][slots] = cseg[s] + cr + 0  # dst local row
            # assemble per-core tensors
            colrel_tile[:, r * CH_R:(r + 1) * CH_R] = crel.T
            for bi in range(BR):
                toks = gtok[bi * (GB // P):(bi + 1) * (GB // P)].reshape(-1)
                gidx[r * BR + bi] = np.tile(
                    toks.reshape(GB // 16, 16).T, (8, 1))
            for pi in range(2):
                sidx[2 * r + pi] = np.tile(
                    sreg[pi].reshape((TPR * P) // 16, 16).T, (8, 1))

        nb = cfg["NB"]
        dloc = np.ones(NPCP, np.float32)
        dloc[:NPC] = dinv[c * NPC:(c + 1) * NPC]
        dinv_pp = dloc.reshape(nb, P).T.copy()          # [128, NB]
        dinv2_pp = (dloc * dloc).reshape(nb, P).T.copy()
        sqdloc = np.ones(NPCP, np.float32)
        sqdloc[:NPC] = np.sqrt(deg[c * NPC:(c + 1) * NPC])

        # x slice for this core, laid out so each phase-Z block load is
        # contiguous per partition: xt[b, p, k*128 + i] = x[base + b*128 + i,
        # k*128 + p]  (partition p = feature-within-ktile, free = (k, node)).
        xc = x[c * NPC:(c + 1) * NPC].astype(ml_dtypes.bfloat16)
        xpad = np.zeros((NPCP, KP), dtype=ml_dtypes.bfloat16)
        xpad[:NPC, :FIN] = xc
        xt = np.ascontiguousarray(
            xpad.reshape(NB, P, cfg["KT"], P).transpose(0, 3, 2, 1)
        ).reshape(NB, P, cfg["KT"] * P)
        # pair up blocks so each phase-Z load is 2 blocks = 6KB/partition
        assert NB % 2 == 0
        xt = np.ascontiguousarray(
            xt.reshape(NB // 2, 2, P, cfg["KT"] * P).transpose(0, 2, 1, 3)
        ).reshape(NB // 2, P, 2 * cfg["KT"] * P)

        m = {
            "xt": xt,
            "w1": w1, "w2": w2, "iota32": iota32, "ident": ident,
            "colrel": colrel_tile.astype(ml_dtypes.bfloat16),
            "gidx": gidx.astype(np.int16),
            "sidx": sidx.astype(np.int16),
            "dinv_pp": dinv_pp, "dinv2_pp": dinv2_pp,
            "b1f": np.tile(b1r, (P, 1)), "b2r": b2r,
            "sqd": sqdloc.reshape(1, NPCP),
            "sqd_pp": sqdloc.reshape(nb, P).T.copy(),
        }
        in_maps.append(m)

    cfg["HAS_B1"] = has_b1
    cfg["HAS_B2"] = has_b2
    return cfg, in_maps


# ------------------------------------------------------------- program build

def _build_program(cfg):
    NB, KT, NPCP, RN, TBL = cfg["NB"], cfg["KT"], cfg["NPCP"], cfg["RN"], cfg["TBL"]
    CH_R, GB, BR, TPR, ARR = cfg["CH_R"], cfg["GB"], cfg["BR"], cfg["TPR"], cfg["ARR"]
    OUT = cfg["OUT"]
    CHUNKS = 4 * CH_R
    SPB = GB // P      # chunk slots per gather batch
    NPAIR = SPB // 8   # psum-tile pairs per batch
    SREG = TPR * P     # scatter indices per (range, parity) region

    nc = bacc.Bacc("TRN2", target_bir_lowering=False, debug=False,
                   num_devices=C, num_swdge_queues=4)

    xt_d = nc.dram_tensor("xt", [NB // 2, P, 2 * KT * P], BF16,
                          kind="ExternalInput")
    w1_d = nc.dram_tensor("w1", [cfg["KP"], 64], BF16, kind="ExternalInput")
    w2_d = nc.dram_tensor("w2", [64, OUT], BF16, kind="ExternalInput")
    iota_d = nc.dram_tensor("iota32", [P, SLOT], BF16, kind="ExternalInput")
    ident_d = nc.dram_tensor("ident", [P, P], BF16, kind="ExternalInput")
    colrel_d = nc.dram_tensor("colrel", [P, CHUNKS], BF16, kind="ExternalInput")
    gidx_d = nc.dram_tensor("gidx", [4 * BR, P, GB // 16], I16, kind="ExternalInput")
    sidx_d = nc.dram_tensor("sidx", [8, P, SREG // 16], I16, kind="ExternalInput")
    dinv_d = nc.dram_tensor("dinv_pp", [P, NB], F32, kind="ExternalInput")
    dinv2_d = nc.dram_tensor("dinv2_pp", [P, NB], F32, kind="ExternalInput")
    b1f_d = nc.dram_tensor("b1f", [P, 64], F32, kind="ExternalInput")
    b2_d = nc.dram_tensor("b2r", [1, OUT], F32, kind="ExternalInput")
    sqd_d = nc.dram_tensor("sqd", [1, NPCP], F32, kind="ExternalInput")
    sqdpp_d = nc.dram_tensor("sqd_pp", [P, NB], F32, kind="ExternalInput")
    out_d = nc.dram_tensor("out", [NPCP, OUT], F32, kind="ExternalOutput")

    with tile.TileContext(nc) as tc:
        with tc.tile_pool(name="const", bufs=1) as cpool, \
             tc.tile_pool(name="sb", bufs=3) as sb, \
             tc.tile_pool(name="parts", bufs=4) as parts_pool, \
             tc.tile_pool(name="psum", bufs=2, space="PSUM") as pp, \
             tc.tile_pool(name="dram", bufs=1, space="DRAM") as dram:

            # ---- constants
            w1_t = cpool.tile([P, KT, 64], BF16)
            nc.sync.dma_start(
                out=w1_t[:], in_=w1_d.ap().rearrange("(k p) e -> p k e", p=P))
            w2_t = cpool.tile([64, OUT], BF16)
            nc.sync.dma_start(out=w2_t[:], in_=w2_d.ap())
            iota_t = cpool.tile([P, SLOT], BF16)
            nc.sync.dma_start(out=iota_t[:], in_=iota_d.ap())
            ident_t = cpool.tile([P, P], BF16)
            nc.sync.dma_start(out=ident_t[:], in_=ident_d.ap())
            colrel_t = cpool.tile([P, CHUNKS], BF16)
            nc.sync.dma_start(out=colrel_t[:], in_=colrel_d.ap())
            dinv_t = cpool.tile([P, NB], F32)
            nc.sync.dma_start(out=dinv_t[:], in_=dinv_d.ap())
            dinv2_t = cpool.tile([P, NB], F32)
            nc.sync.dma_start(out=dinv2_t[:], in_=dinv2_d.ap())
            b2_t = cpool.tile([1, OUT], F32)
            nc.sync.dma_start(out=b2_t[:], in_=b2_d.ap())
            # all gather/scatter index tables live in SBUF for the whole run
            gixt_all = cpool.tile([P, 4 * BR, GB // 16], I16)
            nc.sync.dma_start(
                out=gixt_all[:],
                in_=gidx_d.ap().rearrange("b p g -> p b g"))
            sxt_all = cpool.tile([P, 8, SREG // 16], I16)
            nc.sync.dma_start(
                out=sxt_all[:],
                in_=sidx_d.ap().rearrange("r p g -> p r g"))
            sqd_t = None
            if cfg["HAS_B2"]:
                sqd_t = cpool.tile([1, NPCP], F32)
                nc.sync.dma_start(out=sqd_t[:], in_=sqd_d.ap())
            b1f_t = None
            sqdpp_t = None
            if cfg["HAS_B1"]:
                b1f_t = cpool.tile([P, 64], F32)
                nc.sync.dma_start(out=b1f_t[:], in_=b1f_d.ap())
                sqdpp_t = cpool.tile([P, NB], F32)
                nc.sync.dma_start(out=sqdpp_t[:], in_=sqdpp_d.ap())

            zz = cpool.tile([P, 4096], BF16)
            nc.vector.memset(zz[:], 0.0)

            # ---- DRAM temporaries
            z_loc = dram.tile([NPCP, P], BF16)
            zfull = dram.tile([TBL, P], BF16, addr_space="Shared")
            h1_loc = dram.tile([NPCP, P], BF16)
            h1full = dram.tile([TBL, P], BF16, addr_space="Shared")
            arrA = dram.tile([ARR, P], FP16)
            arrB = dram.tile([ARR, P], FP16)

            def zero_dram(t, rows, dt):
                src = zz[:].bitcast(dt) if dt != BF16 else zz[:]
                off = 0
                while off < rows:
                    n = min(4096, rows - off)
                    nc.sync.dma_start(
                        out=t[:][off:off + n, :].rearrange(
                            "(p a) e -> p (a e)", p=P),
                        in_=src[:, :n],
                    )
                    off += n

            zero_dram(arrA, ARR, FP16)
            zero_dram(arrB, ARR, FP16)

            # ---- phase Z: z_loc = dinv * (x @ W1), node-major bf16
            for bb in range(NB // 2):
                xtt = sb.tile([P, 2, KT, P], BF16, tag="xtt")
                nc.sync.dma_start(out=xtt[:], in_=xt_d.ap()[bb])
                for half in range(2):
                    b = 2 * bb + half
                    psz = pp.tile([P, 64], F32, tag="psz")
                    for k in range(KT):
                        nc.tensor.matmul(
                            out=psz[:], lhsT=xtt[:, half, k, :],
                            rhs=w1_t[:, k, :],
                            start=(k == 0), stop=(k == KT - 1))
                    zst = sb.tile([P, 64], BF16, tag="zst")
                    nc.scalar.activation(
                        out=zst[:], in_=psz[:],
                        func=mybir.ActivationFunctionType.Copy,
                        scale=dinv_t[:, b:b + 1])
                    nc.sync.dma_start(
                        out=z_loc[:][b * P:(b + 1) * P, 0:64], in_=zst[:])

            nc.gpsimd.collective_compute(
                "AllGather", mybir.AluOpType.bypass,
                replica_groups=[list(range(C))],
                ins=[z_loc.opt()], outs=[zfull.opt()])

            # ---- aggregation layer (shared for L1/L2)
            def agg_layer(src_full):
                for r in range(4):
                    part_even = parts_pool.tile([P, TPR, 64], FP16, tag="parts")
                    part_odd = parts_pool.tile([P, TPR, 64], FP16, tag="parts")
                    partials = [part_even, part_odd]
                    for bi in range(BR):
                        bidx = r * BR + bi
                        msg = sb.tile([P, SPB, P], BF16, tag="msg")
                        nc.gpsimd.dma_gather(
                            out_ap=msg[:],
                            in_ap=src_full[:][r * RN:(r + 1) * RN, :],
                            idxs_ap=gixt_all[:, bidx, :],
                            num_idxs=GB, num_idxs_reg=GB, elem_size=P,
                            queue_num=bi % 4)
                        s1t = sb.tile([P, SPB, SLOT], BF16, tag="s1t")
                        cb = bidx * SPB
                        nc.vector.tensor_tensor(
                            out=s1t[:],
                            in0=iota_t[:][:, None, :].to_broadcast([P, SPB, SLOT]),
                            in1=colrel_t[:, cb:cb + SPB][:, :, None].to_broadcast(
                                [P, SPB, SLOT]),
                            op=mybir.AluOpType.is_equal)
                        for pair in range(NPAIR):
                            ps_e = pp.tile([P, 64], F32, tag="pse")
                            ps_o = pp.tile([P, 64], F32, tag="pso")
                            ps = [ps_e, ps_o]
                            for jj in range(8):
                                cl = pair * 8 + jj
                                q = (jj // 2)
                                nc.tensor.matmul(
                                    out=ps[jj % 2][SLOT * q:SLOT * (q + 1), :],
                                    lhsT=s1t[:, cl, :],
                                    rhs=msg[:, cl, 0:64],
                                    start=True, stop=True,
                                    tile_position=(0, SLOT * q),
                                    skip_group_check=True)
                            tr = bi * NPAIR + pair
                            for pi in range(2):
                                if tr % 2 == 0:
                                    nc.vector.tensor_copy(
                                        out=partials[pi][:, tr, :], in_=ps[pi][:])
                                else:
                                    nc.scalar.copy(
                                        out=partials[pi][:, tr, :], in_=ps[pi][:])
                    # scatter-add this range's partial tiles (<=1024 idx per
                    # call; ragged tail allowed).  All scatters into the same
                    # accumulator array stay on one queue (FIFO) so their RMWs
                    # never run concurrently; gathers share the queues freely.
                    for pi in range(2):
                        arr = arrA if pi == 0 else arrB
                        off = 0
                        while off < SREG:
                            n = min(1024, SREG - off)
                            nc.gpsimd.dma_scatter_add(
                                out_ap=arr[:][:, 0:64],
                                in_ap=partials[pi][:, off // P:(off + n) // P, :],
                                idxs_ap=sxt_all[:, 2 * r + pi,
                                                off // 16:(off + n) // 16],
                                num_idxs=n, num_idxs_reg=n,
                                elem_size=64, elem_step=P,
                                queue_num=pi)
                            off += n

            agg_layer(zfull)

            # ---- L1 epilogue:
            #   h1_loc = relu(dinv^2*(agg + z_self) [+ dinv*b1]) (scaled h1)
            for b in range(NB):
                at = sb.tile([P, 64], FP16, tag="at")
                nc.sync.dma_start(out=at[:], in_=arrA[:][b * P:(b + 1) * P, 0:64])
                bt = sb.tile([P, 64], FP16, tag="bt")
                nc.sync.dma_start(out=bt[:], in_=arrB[:][b * P:(b + 1) * P, 0:64])
                zlt = sb.tile([P, 64], BF16, tag="zlt")
                nc.sync.dma_start(
                    out=zlt[:], in_=z_loc[:][b * P:(b + 1) * P, 0:64])
                st = sb.tile([P, 64], F32, tag="st")
                nc.vector.tensor_tensor(
                    out=st[:], in0=at[:], in1=bt[:], op=mybir.AluOpType.add)
                nc.vector.tensor_tensor(
                    out=st[:], in0=st[:], in1=zlt[:], op=mybir.AluOpType.add)
                if cfg["HAS_B1"]:
                    # st += sqrt(deg) * b1 (per-partition scalar x row vector)
                    tmp = sb.tile([P, 64], F32, tag="tmpb")
                    nc.vector.tensor_scalar_mul(
                        tmp[:], b1f_t[:], sqdpp_t[:, b:b + 1])
                    nc.vector.tensor_tensor(
                        out=st[:], in0=st[:], in1=tmp[:], op=mybir.AluOpType.add)
                h1t = sb.tile([P, 64], BF16, tag="h1t")
                nc.scalar.activation(
                    out=h1t[:], in_=st[:],
                    func=mybir.ActivationFunctionType.Relu,
                    scale=dinv2_t[:, b:b + 1])
                nc.sync.dma_start(
                    out=h1_loc[:][b * P:(b + 1) * P, 0:64], in_=h1t[:])

            zero_dram(arrA, ARR, FP16)
            zero_dram(arrB, ARR, FP16)
            nc.gpsimd.collective_compute(
                "AllGather", mybir.AluOpType.bypass,
                replica_groups=[list(range(C))],
                ins=[h1_loc.opt()], outs=[h1full.opt()])

            agg_layer(h1full)

            # ---- L2 epilogue: out = sigmoid(dinv * ((agg2+h_self) @ W2) [+ b2])
            for b in range(NB):
                at = sb.tile([P, 64], FP16, tag="at")
                nc.sync.dma_start(out=at[:], in_=arrA[:][b * P:(b + 1) * P, 0:64])
                bt = sb.tile([P, 64], FP16, tag="bt")
                nc.sync.dma_start(out=bt[:], in_=arrB[:][b * P:(b + 1) * P, 0:64])
                hlt = sb.tile([P, 64], BF16, tag="hlt")
                nc.sync.dma_start(
                    out=hlt[:], in_=h1_loc[:][b * P:(b + 1) * P, 0:64])
                s0 = sb.tile([P, 64], F32, tag="s0")
                nc.vector.tensor_tensor(
                    out=s0[:], in0=at[:], in1=bt[:], op=mybir.AluOpType.add)
                st = sb.tile([P, 64], BF16, tag="st2")
                nc.vector.tensor_tensor(
                    out=st[:], in0=s0[:], in1=hlt[:], op=mybir.AluOpType.add)
                tp = pp.tile([64, P], BF16, tag="psz")
                nc.tensor.transpose(out=tp[:], in_=st[:], identity=ident_t[:])
                zt = sb.tile([64, P], BF16, tag="zt")
                if b % 2 == 0:
                    nc.scalar.copy(out=zt[:], in_=tp[:])
                else:
                    nc.vector.tensor_copy(out=zt[:], in_=tp[:])
                ps3 = pp.tile([P, OUT], F32, tag="pse")
                nc.tensor.matmul(
                    out=ps3[:], lhsT=zt[:], rhs=w2_t[:],
                    start=True, stop=not cfg["HAS_B2"],
                    skip_group_check=True)
                if cfg["HAS_B2"]:
                    nc.tensor.matmul(
                        out=ps3[:], lhsT=sqd_t[:, b * P:(b + 1) * P],
                        rhs=b2_t[:], start=False, stop=True,
                        skip_group_check=True)
                ot = sb.tile([P, OUT], F32, tag="ot")
                nc.scalar.activation(
                    out=ot[:], in_=ps3[:],
                    func=mybir.ActivationFunctionType.Sigmoid,
                    scale=dinv_t[:, b:b + 1])
                nc.sync.dma_start(out=out_d.ap()[b * P:(b + 1) * P, :], in_=ot[:])

    nc.compile()
    return nc


_PROGRAM_CACHE = {}
LAST_EXEC_NS = None
LAST_TRACE = None


def _get_program(cfg):
    key = tuple(sorted((k, v) for k, v in cfg.items()))
    if key not in _PROGRAM_CACHE:
        _PROGRAM_CACHE[key] = _build_program(cfg)
    return _PROGRAM_CACHE[key]


def kernel(x, edge_index, W1, b1, W2, b2):
    x = np.asarray(x, np.float32)
    edge_index = np.asarray(edge_index)
    W1 = np.asarray(W1, np.float32)
    b1 = np.asarray(b1, np.float32)
    W2 = np.asarray(W2, np.float32)
    b2 = np.asarray(b2, np.float32)

    cfg, in_maps = _preprocess(x, edge_index, W1, b1, W2, b2)
    nc = _get_program(cfg)
    trace = bool(os.environ.get("KERNEL_TRACE"))
    res = run_bass_kernel_spmd(nc, in_maps, core_ids=list(range(C)), trace=trace)
    global LAST_EXEC_NS, LAST_TRACE
    if res.exec_time_ns:
        LAST_EXEC_NS = res.exec_time_ns
        LAST_TRACE = res
    NPC, OUT = cfg["NPC"], cfg["OUT"]
    out = np.empty((cfg["N"], OUT), np.float32)
    for c in range(C):
        out[c * NPC:(c + 1) * NPC] = res.results[c]["out"][:NPC]
    return out


# revision 15
# speedup vs baseline: 3.3019x; 1.2033x over previous
"""2-layer GCN (GCNConv -> ReLU -> GCNConv -> Sigmoid) on 8 TRN2 NeuronCores.

Strategy (dst-node sharding, 8 cores):
  - Nodes sharded by destination range: core c owns dst rows [c*NPC, (c+1)*NPC).
  - Fold the symmetric normalization into per-node scales:
        out_d = sigmoid(dinv_d * (A0 @ (dinv*relu(dinv*(A0 @ (dinv*x@W1)) ...)))...
    so the sparse aggregation A0 (unweighted multi-adjacency) acts on 50-wide
    "scaled" tables and no per-edge weight is needed.  Self-loops are NOT sent
    through the edge machinery; their contribution (the node's own scaled row)
    is added locally in each layer epilogue.
  - Per layer: z table (node-major, bf16, rows padded to 256B) is AllGathered
    into a Shared scratchpad; each core gathers z[src] for its edges with
    dma_gather (int16 indices -> 4 gathers against 2-core table ranges),
    reduces 128-edge chunks with one-hot S1 matmuls (S1 built on-device by DVE
    is_equal vs iota), and scatter-adds per-chunk partial sums into fp16 dst
    accumulators with dma_scatter_add (conflict-free by (range, chunk-parity)
    regions split across two accumulator arrays).
  - SWDGE work is spread over 4 queues: gathers round-robin queues 0/1 so
    descriptor prep overlaps ring drain; arrA scatters serialize on queue 2,
    arrB scatters on queue 3 (scatters into the same array must stay ordered
    to avoid concurrent read-modify-write on shared dst rows).
  - Epilogues apply dinv scales/bias/activation on ACT, and the tiny W2 matmul
    runs per dst block after a PE transpose.

Host side does only index/metadata preprocessing (sorting edges, degree
counts, chunk layout) and input re-layout (x transposed + bf16).
"""

import os
import numpy as np
import ml_dtypes

import concourse.bass as bass
import concourse.bacc as bacc
import concourse.tile as tile
import concourse.mybir as mybir
from concourse.bass_utils import run_bass_kernel_spmd

BF16 = mybir.dt.bfloat16
FP16 = mybir.dt.float16
F32 = mybir.dt.float32
I16 = mybir.dt.int16

C = 8        # cores
P = 128      # partitions
SLOT = 32    # dst slots per chunk (chunk spans < 32 dst nodes)
DEAD = SLOT  # col_rel value marking a dead (padded) edge


def _cfg_for(n_nodes, fin, hid, out_dim, ch_r, gb):
    npc = n_nodes // C
    nb = -(-npc // P)
    npcp = nb * P
    kt = -(-fin // P)
    cfg = dict(
        N=n_nodes, FIN=fin, HID=hid, OUT=out_dim,
        NPC=npc, NB=nb, NPCP=npcp, KT=kt, KP=kt * P,
        RN=2 * npcp,                  # rows per gather range (2 cores)
        TBL=C * npcp,                 # allgathered table rows
        CH_R=ch_r,                    # chunks per (core, range), uniform
        GB=gb,                        # gather batch tokens
        BR=(ch_r * P) // gb,          # gather batches per range
        TPR=ch_r // 8,                # partial tiles per (range, parity) region
        ARR=npcp + P,                 # accumulator rows (+dummy block)
    )
    assert cfg["BR"] * gb == ch_r * P and ch_r % 8 == 0 and gb % 128 == 0
    return cfg


# ----------------------------------------------------------------- host prep

def _preprocess(x, edge_index, W1, b1, W2, b2):
    N, FIN = x.shape
    HID = W1.shape[1]
    OUT = W2.shape[1]
    assert N % C == 0
    NPC = N // C
    NB = -(-NPC // P)
    NPCP = NB * P
    RN = 2 * NPCP

    rows = edge_index[0].astype(np.int64)
    cols = edge_index[1].astype(np.int64)

    # degree includes the self-loop (GCNConv add_self_loops=True)
    deg = (np.bincount(cols, minlength=N) + 1).astype(np.float32)
    dinv = (1.0 / np.sqrt(deg.astype(np.float64))).astype(np.float32)

    # table row of node n in the allgathered (row-padded) table
    tbl_row = (rows // NPC) * NPCP + (rows % NPC)
    src_range = tbl_row // RN
    idx_local = (tbl_row - src_range * RN).astype(np.int64)
    core = cols // NPC
    col_local = (cols - core * NPC).astype(np.int64)

    order = np.lexsort((col_local, src_range, core))
    core_s = core[order]
    rng_s = src_range[order]
    coll_s = col_local[order]
    idxl_s = idx_local[order]

    # chunk every (core, range) segment: break at 128 tokens or dst span 32
    bounds_all = {}
    max_chunks = 0
    seg_edges = {}
    for c in range(C):
        c_end = np.searchsorted(core_s, c + 1)
        c_start = np.searchsorted(core_s, c)
        for r in range(4):
            s0 = c_start + np.searchsorted(rng_s[c_start:c_end], r)
            s1 = c_start + np.searchsorted(rng_s[c_start:c_end], r + 1)
            seg_edges[(c, r)] = (s0, s1)
            cseg = coll_s[s0:s1]
            bounds = []
            i = 0
            n = len(cseg)
            while i < n:
                j = int(np.searchsorted(cseg, cseg[i] + SLOT, side="left"))
                j = min(j, i + P, n)
                bounds.append((i, j))
                i = j
            bounds_all[(c, r)] = bounds
            max_chunks = max(max_chunks, len(bounds))
    ch_r = max(64, ((max_chunks + 7) // 8) * 8)
    # dma_gather/dma_scatter_add are limited to 1024 indices per instruction
    # (SWDGE descriptor-ring capacity; >1024 wedges the device).
    gb = 1024
    cfg = _cfg_for(N, FIN, HID, OUT, ch_r, gb)
    CH_R, GB, BR, TPR = cfg["CH_R"], cfg["GB"], cfg["BR"], cfg["TPR"]
    CHUNKS = 4 * CH_R
    DUMMY = NPCP  # dummy dst row in accumulator arrays

    # weights / tables, shared across cores
    KP = cfg["KP"]
    w1 = np.zeros((KP, 64), dtype=ml_dtypes.bfloat16)
    w1[:FIN, :HID] = W1.astype(ml_dtypes.bfloat16)
    w2 = np.zeros((64, OUT), dtype=ml_dtypes.bfloat16)
    w2[:HID, :] = W2.astype(ml_dtypes.bfloat16)
    iota32 = np.tile(np.arange(SLOT, dtype=np.float32), (P, 1)).astype(ml_dtypes.bfloat16)
    ident = np.eye(P, dtype=np.float32).astype(ml_dtypes.bfloat16)
    b1r = np.zeros((1, 64), np.float32)
    b1r[0, :HID] = b1
    b2r = b2.reshape(1, OUT).astype(np.float32)
    has_b1 = bool(np.any(b1))
    has_b2 = bool(np.any(b2))

    in_maps = []
    for c in range(C):
        gidx = np.zeros((4 * BR, P, GB // 16), np.int16)
        colrel_tile = np.full((P, CHUNKS), float(DEAD), np.float32)
        sidx = np.full((8, P, (TPR * P) // 16), DUMMY, np.int64)

        for r in range(4):
            s0, s1 = seg_edges[(c, r)]
            cseg = coll_s[s0:s1]
            iseg = idxl_s[s0:s1]
            bounds = bounds_all[(c, r)]
            gtok = np.zeros((CH_R, P), np.int64)
            crel = np.full((CH_R, P), DEAD, np.int64)
            sreg = np.full((2, TPR * P), DUMMY, np.int64)  # per parity
            for j, (s, e) in enumerate(bounds):
                L = e - s
                gtok[j, :L] = iseg[s:e]
                if L < P:
                    gtok[j, L:] = iseg[e - 1]
                cr = cseg[s:e] - cseg[s]
                crel[j, :L] = cr
                # scatter slots: chunk j -> region (r, j%2), tile (j%8)//... :
                pi = j % 2
                tr = j // 8
                q = (j % 8) // 2
                slots = tr * P + q * SLOT + cr
                sreg[pi][slots] = cseg[s] + cr + 0  # dst local row
            # assemble per-core tensors
            colrel_tile[:, r * CH_R:(r + 1) * CH_R] = crel.T
            for bi in range(BR):
                toks = gtok[bi * (GB // P):(bi + 1) * (GB // P)].reshape(-1)
                gidx[r * BR + bi] = np.tile(
                    toks.reshape(GB // 16, 16).T, (8, 1))
            for pi in range(2):
                sidx[2 * r + pi] = np.tile(
                    sreg[pi].reshape((TPR * P) // 16, 16).T, (8, 1))

        nb = cfg["NB"]
        dloc = np.ones(NPCP, np.float32)
        dloc[:NPC] = dinv[c * NPC:(c + 1) * NPC]
        dinv_pp = dloc.reshape(nb, P).T.copy()          # [128, NB]
        dinv2_pp = (dloc * dloc).reshape(nb, P).T.copy()
        sqdloc = np.ones(NPCP, np.float32)
        sqdloc[:NPC] = np.sqrt(deg[c * NPC:(c + 1) * NPC])

        # x slice for this core, laid out so each phase-Z block load is
        # contiguous per partition: xt[b, p, k*128 + i] = x[base + b*128 + i,
        # k*128 + p]  (partition p = feature-within-ktile, free = (k, node)).
        xc = x[c * NPC:(c + 1) * NPC].astype(ml_dtypes.bfloat16)
        xpad = np.zeros((NPCP, KP), dtype=ml_dtypes.bfloat16)
        xpad[:NPC, :FIN] = xc
        xt = np.ascontiguousarray(
            xpad.reshape(NB, P, cfg["KT"], P).transpose(0, 3, 2, 1)
        ).reshape(NB, P, cfg["KT"] * P)
        # pair up blocks so each phase-Z load is 2 blocks = 6KB/partition
        assert NB % 2 == 0
        xt = np.ascontiguousarray(
            xt.reshape(NB // 2, 2, P, cfg["KT"] * P).transpose(0, 2, 1, 3)
        ).reshape(NB // 2, P, 2 * cfg["KT"] * P)

        m = {
            "xt": xt,
            "w1": w1, "w2": w2, "iota32": iota32, "ident": ident,
            "colrel": colrel_tile.astype(ml_dtypes.bfloat16),
            "gidx": gidx.astype(np.int16),
            "sidx": sidx.astype(np.int16),
            "dinv_pp": dinv_pp, "dinv2_pp": dinv2_pp,
            "b1f": np.tile(b1r, (P, 1)), "b2r": b2r,
            "sqd": sqdloc.reshape(1, NPCP),
            "sqd_pp": sqdloc.reshape(nb, P).T.copy(),
        }
        in_maps.append(m)

    cfg["HAS_B1"] = has_b1
    cfg["HAS_B2"] = has_b2
    return cfg, in_maps


# ------------------------------------------------------------- program build

def _build_program(cfg):
    NB, KT, NPCP, RN, TBL = cfg["NB"], cfg["KT"], cfg["NPCP"], cfg["RN"], cfg["TBL"]
    CH_R, GB, BR, TPR, ARR = cfg["CH_R"], cfg["GB"], cfg["BR"], cfg["TPR"], cfg["ARR"]
    OUT = cfg["OUT"]
    CHUNKS = 4 * CH_R
    SPB = GB // P      # chunk slots per gather batch
    NPAIR = SPB // 8   # psum-tile pairs per batch
    SREG = TPR * P     # scatter indices per (range, parity) region

    nc = bacc.Bacc("TRN2", target_bir_lowering=False, debug=False,
                   num_devices=C, num_swdge_queues=4)

    xt_d = nc.dram_tensor("xt", [NB // 2, P, 2 * KT * P], BF16,
                          kind="ExternalInput")
    w1_d = nc.dram_tensor("w1", [cfg["KP"], 64], BF16, kind="ExternalInput")
    w2_d = nc.dram_tensor("w2", [64, OUT], BF16, kind="ExternalInput")
    iota_d = nc.dram_tensor("iota32", [P, SLOT], BF16, kind="ExternalInput")
    ident_d = nc.dram_tensor("ident", [P, P], BF16, kind="ExternalInput")
    colrel_d = nc.dram_tensor("colrel", [P, CHUNKS], BF16, kind="ExternalInput")
    gidx_d = nc.dram_tensor("gidx", [4 * BR, P, GB // 16], I16, kind="ExternalInput")
    sidx_d = nc.dram_tensor("sidx", [8, P, SREG // 16], I16, kind="ExternalInput")
    dinv_d = nc.dram_tensor("dinv_pp", [P, NB], F32, kind="ExternalInput")
    dinv2_d = nc.dram_tensor("dinv2_pp", [P, NB], F32, kind="ExternalInput")
    b1f_d = nc.dram_tensor("b1f", [P, 64], F32, kind="ExternalInput")
    b2_d = nc.dram_tensor("b2r", [1, OUT], F32, kind="ExternalInput")
    sqd_d = nc.dram_tensor("sqd", [1, NPCP], F32, kind="ExternalInput")
    sqdpp_d = nc.dram_tensor("sqd_pp", [P, NB], F32, kind="ExternalInput")
    out_d = nc.dram_tensor("out", [NPCP, OUT], F32, kind="ExternalOutput")

    with tile.TileContext(nc) as tc:
        with tc.tile_pool(name="const", bufs=1) as cpool, \
             tc.tile_pool(name="sb", bufs=4) as sb, \
             tc.tile_pool(name="parts", bufs=4) as parts_pool, \
             tc.tile_pool(name="psum", bufs=2, space="PSUM") as pp, \
             tc.tile_pool(name="dram", bufs=1, space="DRAM") as dram:

            # ---- constants
            w1_t = cpool.tile([P, KT, 64], BF16)
            nc.sync.dma_start(
                out=w1_t[:], in_=w1_d.ap().rearrange("(k p) e -> p k e", p=P))
            w2_t = cpool.tile([64, OUT], BF16)
            nc.sync.dma_start(out=w2_t[:], in_=w2_d.ap())
            iota_t = cpool.tile([P, SLOT], BF16)
            nc.sync.dma_start(out=iota_t[:], in_=iota_d.ap())
            ident_t = cpool.tile([P, P], BF16)
            nc.sync.dma_start(out=ident_t[:], in_=ident_d.ap())
            colrel_t = cpool.tile([P, CHUNKS], BF16)
            nc.sync.dma_start(out=colrel_t[:], in_=colrel_d.ap())
            dinv_t = cpool.tile([P, NB], F32)
            nc.sync.dma_start(out=dinv_t[:], in_=dinv_d.ap())
            dinv2_t = cpool.tile([P, NB], F32)
            nc.sync.dma_start(out=dinv2_t[:], in_=dinv2_d.ap())
            b2_t = cpool.tile([1, OUT], F32)
            nc.sync.dma_start(out=b2_t[:], in_=b2_d.ap())
            # all gather/scatter index tables live in SBUF for the whole run
            gixt_all = cpool.tile([P, 4 * BR, GB // 16], I16)
            nc.sync.dma_start(
                out=gixt_all[:],
                in_=gidx_d.ap().rearrange("b p g -> p b g"))
            sxt_all = cpool.tile([P, 8, SREG // 16], I16)
            nc.sync.dma_start(
                out=sxt_all[:],
                in_=sidx_d.ap().rearrange("r p g -> p r g"))
            sqd_t = None
            if cfg["HAS_B2"]:
                sqd_t = cpool.tile([1, NPCP], F32)
                nc.sync.dma_start(out=sqd_t[:], in_=sqd_d.ap())
            b1f_t = None
            sqdpp_t = None
            if cfg["HAS_B1"]:
                b1f_t = cpool.tile([P, 64], F32)
                nc.sync.dma_start(out=b1f_t[:], in_=b1f_d.ap())
                sqdpp_t = cpool.tile([P, NB], F32)
                nc.sync.dma_start(out=sqdpp_t[:], in_=sqdpp_d.ap())

            zz = cpool.tile([P, 4096], BF16)
            nc.vector.memset(zz[:], 0.0)

            # ---- DRAM temporaries
            z_loc = dram.tile([NPCP, P], BF16)
            zfull = dram.tile([TBL, P], BF16, addr_space="Shared")
            h1_loc = dram.tile([NPCP, P], BF16)
            h1full = dram.tile([TBL, P], BF16, addr_space="Shared")
            arrA = dram.tile([ARR, P], FP16)
            arrB = dram.tile([ARR, P], FP16)

            def zero_dram(t, rows, dt):
                src = zz[:].bitcast(dt) if dt != BF16 else zz[:]
                off = 0
                while off < rows:
                    n = min(4096, rows - off)
                    nc.sync.dma_start(
                        out=t[:][off:off + n, :].rearrange(
                            "(p a) e -> p (a e)", p=P),
                        in_=src[:, :n],
                    )
                    off += n

            zero_dram(arrA, ARR, FP16)
            zero_dram(arrB, ARR, FP16)

            # ---- phase Z: z_loc = dinv * (x @ W1), node-major bf16
            for bb in range(NB // 2):
                xtt = sb.tile([P, 2, KT, P], BF16, tag="xtt")
                eng = nc.sync if bb % 2 == 0 else nc.scalar
                eng.dma_start(out=xtt[:], in_=xt_d.ap()[bb])
                for half in range(2):
                    b = 2 * bb + half
                    psz = pp.tile([P, 64], F32, tag="psz")
                    for k in range(KT):
                        nc.tensor.matmul(
                            out=psz[:], lhsT=xtt[:, half, k, :],
                            rhs=w1_t[:, k, :],
                            start=(k == 0), stop=(k == KT - 1))
                    zst = sb.tile([P, 64], BF16, tag="zst")
                    nc.scalar.activation(
                        out=zst[:], in_=psz[:],
                        func=mybir.ActivationFunctionType.Copy,
                        scale=dinv_t[:, b:b + 1])
                    nc.sync.dma_start(
                        out=z_loc[:][b * P:(b + 1) * P, 0:64], in_=zst[:])

            nc.gpsimd.collective_compute(
                "AllGather", mybir.AluOpType.bypass,
                replica_groups=[list(range(C))],
                ins=[z_loc.opt()], outs=[zfull.opt()])

            # ---- aggregation layer (shared for L1/L2)
            def agg_layer(src_full):
                for r in range(4):
                    part_even = parts_pool.tile([P, TPR, 64], FP16, tag="parts")
                    part_odd = parts_pool.tile([P, TPR, 64], FP16, tag="parts")
                    partials = [part_even, part_odd]
                    for bi in range(BR):
                        bidx = r * BR + bi
                        msg = sb.tile([P, SPB, P], BF16, tag="msg")
                        nc.gpsimd.dma_gather(
                            out_ap=msg[:],
                            in_ap=src_full[:][r * RN:(r + 1) * RN, :],
                            idxs_ap=gixt_all[:, bidx, :],
                            num_idxs=GB, num_idxs_reg=GB, elem_size=P,
                            queue_num=bi % 4)
                        s1t = sb.tile([P, SPB, SLOT], BF16, tag="s1t")
                        cb = bidx * SPB
                        nc.vector.tensor_tensor(
                            out=s1t[:],
                            in0=iota_t[:][:, None, :].to_broadcast([P, SPB, SLOT]),
                            in1=colrel_t[:, cb:cb + SPB][:, :, None].to_broadcast(
                                [P, SPB, SLOT]),
                            op=mybir.AluOpType.is_equal)
                        for pair in range(NPAIR):
                            ps_e = pp.tile([P, 64], F32, tag="pse")
                            ps_o = pp.tile([P, 64], F32, tag="pso")
                            ps = [ps_e, ps_o]
                            for jj in range(8):
                                cl = pair * 8 + jj
                                q = (jj // 2)
                                nc.tensor.matmul(
                                    out=ps[jj % 2][SLOT * q:SLOT * (q + 1), :],
                                    lhsT=s1t[:, cl, :],
                                    rhs=msg[:, cl, 0:64],
                                    start=True, stop=True,
                                    tile_position=(0, SLOT * q),
                                    skip_group_check=True)
                            tr = bi * NPAIR + pair
                            for pi in range(2):
                                if tr % 2 == 0:
                                    nc.vector.tensor_copy(
                                        out=partials[pi][:, tr, :], in_=ps[pi][:])
                                else:
                                    nc.scalar.copy(
                                        out=partials[pi][:, tr, :], in_=ps[pi][:])
                    # scatter-add this range's partial tiles (<=1024 idx per
                    # call; ragged tail allowed).  All scatters into the same
                    # accumulator array stay on one queue (FIFO) so their RMWs
                    # never run concurrently; gathers share the queues freely.
                    for pi in range(2):
                        arr = arrA if pi == 0 else arrB
                        off = 0
                        while off < SREG:
                            n = min(1024, SREG - off)
                            nc.gpsimd.dma_scatter_add(
                                out_ap=arr[:][:, 0:64],
                                in_ap=partials[pi][:, off // P:(off + n) // P, :],
                                idxs_ap=sxt_all[:, 2 * r + pi,
                                                off // 16:(off + n) // 16],
                                num_idxs=n, num_idxs_reg=n,
                                elem_size=64, elem_step=P,
                                queue_num=pi)
                            off += n

            agg_layer(zfull)

            # ---- L1 epilogue:
            #   h1_loc = relu(dinv^2*(agg + z_self) [+ dinv*b1]) (scaled h1)
            for b in range(NB):
                sfx = str(b % 2)
                at = sb.tile([P, 64], FP16, tag="at" + sfx)
                nc.sync.dma_start(out=at[:], in_=arrA[:][b * P:(b + 1) * P, 0:64])
                bt = sb.tile([P, 64], FP16, tag="bt" + sfx)
                nc.scalar.dma_start(out=bt[:], in_=arrB[:][b * P:(b + 1) * P, 0:64])
                zlt = sb.tile([P, 64], BF16, tag="zlt" + sfx)
                nc.sync.dma_start(
                    out=zlt[:], in_=z_loc[:][b * P:(b + 1) * P, 0:64])
                st = sb.tile([P, 64], F32, tag="st" + sfx)
                nc.vector.tensor_tensor(
                    out=st[:], in0=at[:], in1=bt[:], op=mybir.AluOpType.add)
                nc.vector.tensor_tensor(
                    out=st[:], in0=st[:], in1=zlt[:], op=mybir.AluOpType.add)
                if cfg["HAS_B1"]:
                    # st += sqrt(deg) * b1 (per-partition scalar x row vector)
                    tmp = sb.tile([P, 64], F32, tag="tmpb")
                    nc.vector.tensor_scalar_mul(
                        tmp[:], b1f_t[:], sqdpp_t[:, b:b + 1])
                    nc.vector.tensor_tensor(
                        out=st[:], in0=st[:], in1=tmp[:], op=mybir.AluOpType.add)
                h1t = sb.tile([P, 64], BF16, tag="h1t" + sfx)
                nc.scalar.activation(
                    out=h1t[:], in_=st[:],
                    func=mybir.ActivationFunctionType.Relu,
                    scale=dinv2_t[:, b:b + 1])
                nc.sync.dma_start(
                    out=h1_loc[:][b * P:(b + 1) * P, 0:64], in_=h1t[:])

            zero_dram(arrA, ARR, FP16)
            zero_dram(arrB, ARR, FP16)
            nc.gpsimd.collective_compute(
                "AllGather", mybir.AluOpType.bypass,
                replica_groups=[list(range(C))],
                ins=[h1_loc.opt()], outs=[h1full.opt()])

            agg_layer(h1full)

            # ---- L2 epilogue: out = sigmoid(dinv * ((agg2+h_self) @ W2) [+ b2])
            for b in range(NB):
                sfx = str(b % 2)
                at = sb.tile([P, 64], FP16, tag="at" + sfx)
                nc.sync.dma_start(out=at[:], in_=arrA[:][b * P:(b + 1) * P, 0:64])
                bt = sb.tile([P, 64], FP16, tag="bt" + sfx)
                nc.scalar.dma_start(out=bt[:], in_=arrB[:][b * P:(b + 1) * P, 0:64])
                hlt = sb.tile([P, 64], BF16, tag="hlt" + sfx)
                nc.sync.dma_start(
                    out=hlt[:], in_=h1_loc[:][b * P:(b + 1) * P, 0:64])
                s0 = sb.tile([P, 64], F32, tag="s0" + sfx)
                nc.vector.tensor_tensor(
                    out=s0[:], in0=at[:], in1=bt[:], op=mybir.AluOpType.add)
                st = sb.tile([P, 64], BF16, tag="st2" + sfx)
                nc.vector.tensor_tensor(
                    out=st[:], in0=s0[:], in1=hlt[:], op=mybir.AluOpType.add)
                tp = pp.tile([64, P], BF16, tag="psz" if b % 2 == 0 else "pso")
                nc.tensor.transpose(out=tp[:], in_=st[:], identity=ident_t[:])
                zt = sb.tile([64, P], BF16, tag="zt" + sfx)
                if b % 2 == 0:
                    nc.scalar.copy(out=zt[:], in_=tp[:])
                else:
                    nc.vector.tensor_copy(out=zt[:], in_=tp[:])
                ps3 = pp.tile([P, OUT], F32, tag="pse")
                nc.tensor.matmul(
                    out=ps3[:], lhsT=zt[:], rhs=w2_t[:],
                    start=True, stop=not cfg["HAS_B2"],
                    skip_group_check=True)
                if cfg["HAS_B2"]:
                    nc.tensor.matmul(
                        out=ps3[:], lhsT=sqd_t[:, b * P:(b + 1) * P],
                        rhs=b2_t[:], start=False, stop=True,
                        skip_group_check=True)
                ot = sb.tile([P, OUT], F32, tag="ot" + sfx)
                nc.scalar.activation(
                    out=ot[:], in_=ps3[:],
                    func=mybir.ActivationFunctionType.Sigmoid,
                    scale=dinv_t[:, b:b + 1])
                nc.sync.dma_start(out=out_d.ap()[b * P:(b + 1) * P, :], in_=ot[:])

    nc.compile()
    return nc


_PROGRAM_CACHE = {}
LAST_EXEC_NS = None
LAST_TRACE = None


def _get_program(cfg):
    key = tuple(sorted((k, v) for k, v in cfg.items()))
    if key not in _PROGRAM_CACHE:
        _PROGRAM_CACHE[key] = _build_program(cfg)
    return _PROGRAM_CACHE[key]


def kernel(x, edge_index, W1, b1, W2, b2):
    x = np.asarray(x, np.float32)
    edge_index = np.asarray(edge_index)
    W1 = np.asarray(W1, np.float32)
    b1 = np.asarray(b1, np.float32)
    W2 = np.asarray(W2, np.float32)
    b2 = np.asarray(b2, np.float32)

    cfg, in_maps = _preprocess(x, edge_index, W1, b1, W2, b2)
    nc = _get_program(cfg)
    trace = bool(os.environ.get("KERNEL_TRACE"))
    res = run_bass_kernel_spmd(nc, in_maps, core_ids=list(range(C)), trace=trace)
    global LAST_EXEC_NS, LAST_TRACE
    if res.exec_time_ns:
        LAST_EXEC_NS = res.exec_time_ns
        LAST_TRACE = res
    NPC, OUT = cfg["NPC"], cfg["OUT"]
    out = np.empty((cfg["N"], OUT), np.float32)
    for c in range(C):
        out[c * NPC:(c + 1) * NPC] = res.results[c]["out"][:NPC]
    return out
